# revision 44
# baseline (speedup 1.0000x reference)
"""CropProposals v4: shuffle-free layered stage-2 + DVE fold-assisted stage-1.

Sharding: core k handles batch b = k//2 and a balanced half of its 64
proposals (~32), ALL 64 channels.  fm per core is pre-transposed on host to
[128 = (dm2:2, c:64), 12*576 = (dq2, h, w)] bf16 so multi-slab input DMAs
are 3-dim APs with large contiguous elements (9 DMAs over the SP/Act/Pool
queues, first slabs visible ~2.4us, all by ~3.9us).

Layouts (partition dim first):
  A  [128 = (dm2, c), free (dq2:12, h:24, w:24)]
  Bt [128 = (dm2, c), free (j':12, hb:2, wb:2, pi:PCAP)]  stage-1 results,
      j' normalized per proposal (j' = dq2 - sd//2)
  W  [128, scratch]   DVE fold scratch
  U[k] [64 = (c), free (db:2, hb:2, wb:2, pi:PCAP)]  d-slice layer k
  V  [64 = (c), same free layout]                    final output

Stage 1 (DVE): fused TensorReduce over (lh x lw) keeping (j', hb, wb).
For big crops with even nw (or nh) one half-fold tensor_tensor level (bf16
2x mode, 0.52 ns/elem vs TensorReduce's mode-less 1.04) runs first, then a
smaller TensorReduce.  1x1-bin crops are plain Pool copies.  Execution
order: input-arrival gate, then big-ld first.

Stage 2 (no DMA shuffle; walrus only allows tensor-tensor max on DVE, and
only with equal input partition bases): a d-bin of a proposal maps to one
d-slice per (bin, k) with lane dm2=(r+s2)%2, so Pool stages the k-th slice
of every (run, bin) into layer tile U[k] at partition base 0 (cross-base
1-input copies are legal); slice 0 goes straight into V.  pi columns are
sorted by ascending ld = (nd+1)//2 in three chunks [small-ld early |
big-ld | late small-ld (tail store region)], and ONE in-place DVE
tensor_tensor per level k accumulates U[k] into V over the contiguous
SUFFIX of proposals with ld > k -- one instruction per level for the whole
core amortizes DVE's ~60ns fixed cost, and high levels touch only the
trailing columns.  The output leaves in a bulk store plus a tiny tail
store covering the last TAILN columns (the late chunk).

The host-side planner (_plan_core) models engine clocks and emits all ops
in modeled start order; the tile framework infers exact semaphores from
program order.  DMAs stay outside the per-core branch arms (conditional
DMAs break walrus queue-sem watermarks).

bf16 end-to-end: max() commutes with monotone rounding, so the result
equals round_bf16(exact f32 result); rel err <= 2^-8 ~ 4e-3 within the
2e-2 gate.
"""

import os
import sys

import numpy as np

for _p in ("/opt/trn_rl_repo", os.path.expanduser("~/.axon_site/_ro/trn_rl_repo")):
    if os.path.isdir(_p) and _p not in sys.path:
        sys.path.insert(0, _p)

import ml_dtypes  # noqa: E402

import concourse.bass as bass  # noqa: E402
import concourse.tile as tile  # noqa: E402
import concourse.tile_sem_assignment as _tsa  # noqa: E402
from concourse import mybir  # noqa: E402
from concourse.bass_utils import run_bass_kernel_spmd  # noqa: E402

# Walrus encodes at most ONE sync-wait per compute instruction.  Pin HWDGE
# completion-sem lanes by issuing engine so every consumer waits on one
# per-queue FIFO sem; extra waits are split off by _hoist_extra_waits.
if not getattr(_tsa, "_ant_engine_keyed_lanes", False):
    _orig_assign_tick = _tsa.TileClockTick._assign_tick

    def _assign_tick_engine_lanes(self, inst):
        if isinstance(inst, _tsa.DMAInst) and inst.engine in (
            mybir.EngineType.SP,
            mybir.EngineType.Activation,
        ):
            self.next_hw_dma_idx = 0 if inst.engine == mybir.EngineType.SP else 1
        return _orig_assign_tick(self, inst)

    _tsa.TileClockTick._assign_tick = _assign_tick_engine_lanes
    _tsa._ant_engine_keyed_lanes = True

B, P, C, S = 4, 64, 64, 24
JS = 12
BF = mybir.dt.bfloat16

# cost-model constants (ns)
DVE_EL = 1.0417
DVE_FIX = 60.4
DVE_EL2 = 0.5208     # bf16 packed tensor_tensor (2x mode)
POOL_CP = 1.389      # Pool tensor_copy ns/elem
POOL_OP = 3.0
SEM_HOP = 100.0
CHUNK_CUTS = (500.0,)  # B/C boundary, ns before the last s1 finish
TAILN = 8            # pi columns in the tiny trailing store
LD_A = 4             # ld >= LD_A proposals form the early long-chain chunk

# input DMA plan: per queue, list of (first_slab, n_slabs)
IN_PLAN = {
    "SP": [(0, 1), (3, 2), (9, 1)],
    "Act": [(1, 1), (5, 2), (10, 1)],
    "Pool": [(2, 1), (7, 2), (11, 1)],
}


def _bin_params(corners: np.ndarray):
    """Replicate the reference's float32 bin math. Returns lo, n int64 [B,P,3]
    with axis order (d, h, w)."""
    c = corners.astype(np.float32) / np.float32(4.0)
    LL = np.clip(c[:, :, 0, :], np.float32(0.0), np.float32(21.0))
    UR = c[:, :, 1, :]
    UR = np.where(UR - LL >= np.float32(2.0), UR, LL + np.float32(2.0))
    UR = np.clip(UR, np.float32(2.0), np.float32(23.0))
    lo = np.floor(LL).astype(np.int64)
    n = np.floor(UR).astype(np.int64) - lo
    return lo, n


def _geom(lo_b, n_b, p):
    nd, nh, nw = (int(n_b[p, a]) for a in range(3))
    sd, sh, sw = (int(lo_b[p, a]) for a in range(3))
    lh, lw = (nh + 1) // 2, (nw + 1) // 2
    dq0 = sd // 2
    ndq = (sd + nd - 1) // 2 - dq0 + 1
    return nd, nh, nw, sd, sh, sw, lh, lw, dq0, ndq


def _dma_cost(n_slabs):
    tr = 128 / 16.0 * (n_slabs * 1152 / 22.5)
    return max(500.0, tr + 70.0)


def _slab_vis():
    delays = {"SP": 1717.0, "Act": 1717.0, "Pool": 1883.0}
    t0 = {"SP": 200.0, "Act": 200.0, "Pool": 100.0}
    vis = {}
    for q, dmas in IN_PLAN.items():
        t = t0[q]
        for (s0, cnt) in dmas:
            t += _dma_cost(cnt)
            for s in range(s0, s0 + cnt):
                vis[s] = t + delays[q]
    assert sorted(vis) == list(range(JS)), vis
    return vis


def _pool_busy_until():
    return 100.0 + sum(_dma_cost(c) for _, c in IN_PLAN["Pool"])


def _s1_kind(g):
    """Returns (kind, modeled ns).  kind: 'cp' (Pool), 'tr', 'foldy', 'foldx'."""
    nd, nh, nw, sd, sh, sw, lh, lw, dq0, ndq = g
    K = ndq * 4 * lh * lw
    if lh == 1 and lw == 1:
        return "cp", max(POOL_OP, ndq * 4 * POOL_CP)
    ni = 2 if (ndq > 1 and lh > 1 and lw > 1) else 1
    best = ("tr", ni * DVE_FIX + K * DVE_EL)
    if nw % 2 == 0 and lh > 1 and lw > 1:
        hy = (lh + 1) // 2
        c = 2 * (DVE_FIX + ndq * hy * nw * DVE_EL2) \
            + 2 * (DVE_FIX + ndq * 2 * hy * lw * DVE_EL)
        if c < best[1]:
            best = ("foldy", c)
    if nh % 2 == 0 and lh > 1 and lw > 1:
        hx = (lw + 1) // 2
        c = 2 * (DVE_FIX + ndq * nh * hx * DVE_EL2) \
            + 2 * (DVE_FIX + ndq * 2 * lh * hx * DVE_EL)
        if c < best[1]:
            best = ("foldx", c)
    return best


def _ld_of(nd):
    return (nd + 1) // 2


def _lane_slots(nd, s2, db):
    """[(lane, j0, nj)] with nj>0 for bin db of a (nd, s2) class."""
    ld = _ld_of(nd)
    a = db * (nd // 2)
    b = a + ld
    out = []
    for l in (0, 1):
        r0 = a + ((l - s2 - a) % 2)
        if r0 < b:
            nj = (b - r0 + 1) // 2
            out.append((l, (r0 - l + s2) // 2, nj))
    return out


def _slice_of(nd, s2, db, k):
    """(lane, j') of the k-th d-slice of bin db."""
    r = db * (nd // 2) + k
    l = (r + s2) % 2
    return l, (r - l + s2) // 2


def _pi_order(lo_b, n_b, props, chunk_of=None):
    # ascending ld within a chunk: accumulation level k covers the SUFFIX
    # of proposals with ld > k, so each chunk's last-written columns sit at
    # its end (next to the store-tail region).
    key = lambda p: ((chunk_of[p] if chunk_of else 0),
                     _ld_of(int(n_b[p, 0])), int(n_b[p, 0]),
                     int(lo_b[p, 0]) % 2, int(lo_b[p, 0]), p)
    return sorted(props, key=key)


_split_cache = {}


def _split_assign(lo_b, n_b):
    """Split 64 proposals into two halves minimizing the max modeled end."""
    key = (lo_b.tobytes(), n_b.tobytes())
    if key in _split_cache:
        return _split_cache[key]
    costs = [(_s1_kind(_geom(lo_b, n_b, p))[1], p) for p in range(P)]
    costs.sort(key=lambda t: (-t[0], t[1]))
    halves, tot = ([], []), [0.0, 0.0]
    for c, p in costs:
        h = 0 if tot[0] <= tot[1] else 1
        halves[h].append(p)
        tot[h] += c

    def end(props):
        return _plan_core(lo_b, n_b, props, P)["model_end"]

    e = [end(halves[0]), end(halves[1])]
    for _ in range(8):
        d = 0 if e[0] >= e[1] else 1
        best = (max(e), None)
        for p in halves[d]:
            h0 = [x for x in halves[d] if x != p]
            h1 = halves[1 - d] + [p]
            ne = (end(h0), end(h1))
            if max(ne) < best[0] - 1.0:
                best = (max(ne), (p, ne))
        if best[1] is None:
            break
        p, ne = best[1]
        halves[d].remove(p)
        halves[1 - d].append(p)
        e = [ne[0], ne[1]] if d == 0 else [ne[1], ne[0]]
    res = (sorted(halves[0]), sorted(halves[1]))
    _split_cache[key] = res
    return res


def _runs_of(props, lo_b, n_b):
    i = 0
    while i < len(props):
        nd = int(n_b[props[i], 0])
        s2 = int(lo_b[props[i], 0]) % 2
        m = 1
        while (i + m < len(props)
               and int(n_b[props[i + m], 0]) == nd
               and int(lo_b[props[i + m], 0]) % 2 == s2):
            m += 1
        yield i, m, nd, s2
        i += m


def _plan_core(lo_b, n_b, props, pcap):
    """Schedule one core.  Returns pi columns and one merged ordered op list."""
    vis = _slab_vis()
    geom = {p: _geom(lo_b, n_b, p) for p in props}

    def gate(p):
        g = geom[p]
        dq0, ndq = g[8], g[9]
        return max(vis[q] for q in range(dq0, dq0 + ndq))

    kind = {p: _s1_kind(geom[p]) for p in props}
    # stage-1 priority: arrival gate, then big-ld first (so the trailing
    # accumulation chains belong to small-ld chunks), then big cost first
    order = sorted(props, key=lambda p: (gate(p),
                                         -_ld_of(int(n_b[p, 0])),
                                         -kind[p][1]))
    TD, TP = 0.0, _pool_busy_until()
    s1_fin = {}
    s1_sched = []
    for p in order:
        k, c = kind[p]
        if k == "cp":
            st = max(TP, gate(p))
            TP = st + c
            s1_fin[p] = TP
        else:
            st = max(TD, gate(p))
            TD = st + c
            s1_fin[p] = TD
        s1_sched.append((st, p, k))

    # chunks: 0 = small-ld early, 1 = big-ld (placed after so ld ascends
    # over chunks 0+1 and ONE merged suffix-level chain serves both; its
    # high levels touch only the trailing columns next to the tail store),
    # 2 = last-finishing small-ld proposals, capped to the tail region.
    fmax = max(s1_fin.values())
    cut = CHUNK_CUTS[-1]

    def _chunk(p):
        if _ld_of(int(n_b[p, 0])) >= LD_A:
            return 1
        return 2 if s1_fin[p] > fmax - cut else 0
    chunk_of = {p: _chunk(p) for p in props}
    last = sorted([p for p in props if chunk_of[p] == 2],
                  key=lambda p: s1_fin[p])
    for p in last[:-TAILN] if len(last) > TAILN else []:
        chunk_of[p] = 0
    ncut = 2
    if all(v == ncut for v in chunk_of.values()):
        chunk_of = {p: 0 for p in props}
    pi_order = _pi_order(lo_b, n_b, props, chunk_of)
    # pi columns: front chunks pack from 0; the last chunk is pinned to the
    # final TAILN-capped columns so the split store's tail is uniform.
    n_last = sum(1 for p in props if chunk_of[p] == ncut)
    n_front = len(props) - n_last
    col = {}
    for i, p in enumerate(pi_order):
        col[p] = i if i < n_front else pcap - n_last + (i - n_front)
    pi_of = col

    dve_ops = []
    pool_ops = []
    for st, p, k in s1_sched:
        if k == "cp":
            pool_ops.append((st, dict(kind="s1cp", p=p, pi=pi_of[p])))
        else:
            dve_ops.append((st, dict(kind="s1", p=p, pi=pi_of[p], s1kind=k)))

    # ---- stage-2 ----
    rinfo = []
    i = 0
    while i < len(pi_order):
        nd = int(n_b[pi_order[i], 0])
        s2 = int(lo_b[pi_order[i], 0]) % 2
        m = 1
        while (i + m < len(pi_order)
               and int(n_b[pi_order[i + m], 0]) == nd
               and int(lo_b[pi_order[i + m], 0]) % 2 == s2
               and col[pi_order[i + m]] == col[pi_order[i]] + m):
            m += 1
        members = pi_order[i:i + m]
        hop = SEM_HOP if any(kind[p][0] != "cp" for p in members) else 0.0
        ready = max(s1_fin[p] for p in members) + hop
        rinfo.append(dict(i0=col[pi_order[i]], m=m, nd=nd, s2=s2,
                          ld=_ld_of(nd), ready=ready,
                          chunk=chunk_of[pi_order[i]]))
        i += m

    # Pool slice copies: k-th d-slice of each (run, bin); k==0 goes straight
    # to V, k>=1 to layer tile U[k].  Track per-(chunk, k) completion.
    layer_done = {}
    for r in sorted(rinfo, key=lambda r: r["ready"]):
        i0, m, nd, s2, ld = r["i0"], r["m"], r["nd"], r["s2"], r["ld"]
        c1 = max(POOL_OP, m * 4 * POOL_CP)
        for k in range(ld):
            for db in range(2):
                l, j = _slice_of(nd, s2, db, k)
                st = max(TP, r["ready"])
                TP = st + c1
                pool_ops.append((st, dict(kind="slice", i0=i0, m=m, db=db,
                                          k=k, lane=l, j0=j)))
            key = (r["chunk"], k)
            layer_done[key] = max(layer_done.get(key, 0.0), TP)

    # DVE level accumulations: one merged chain over chunks 0+1 (ld ascends
    # across both), a separate short chain for the tail chunk 2.  Level k
    # covers the SUFFIX of proposals with ld > k.
    for cids in ((0, 1), (2,)):
        cprops = [p for p in pi_order if chunk_of[p] in cids]
        if not cprops:
            continue
        cend = max(col[p] for p in cprops) + 1
        lds = [_ld_of(int(n_b[p, 0])) for p in cprops]
        ldmax = max(lds)
        prev = 0.0
        for k in range(1, ldmax):
            span = sum(1 for x in lds if x > k)
            rel = prev
            for cid in cids:
                rel = max(rel, layer_done.get((cid, k), 0.0) + SEM_HOP,
                          layer_done.get((cid, 0), 0.0) + SEM_HOP)
            st = max(TD, rel)
            TD = st + DVE_FIX + span * 8 * DVE_EL2
            prev = TD
            dve_ops.append((st, dict(kind="acc", g0=cend - span, span=span,
                                     k=k)))
    model_end = max(TD, TP)
    # Merge to one stream ordered by modeled start: tile deps are inferred
    # from program order, so every consumer must be emitted after its
    # producer (consumer start >= producer finish guarantees this).
    s1k = ("s1", "s1cp")
    allops = [(st, 0 if o["kind"] in s1k else 1, i, o)
              for i, (st, o) in enumerate(dve_ops + pool_ops)]
    allops.sort(key=lambda t: (t[0], t[1], t[2]))
    return dict(columns={p: col[p] for p in props}, pi_order=pi_order,
                ops=[o for _, _, _, o in allops],
                n_front=n_front, n_last=n_last,
                nprops=len(props), model_end=model_end)


def _mk_ap(base_ap, extra_offset: int, dims):
    ap = base_ap.copy()
    ap.ap = mybir.VecI64Pair([list(base_ap.ap[0])] + [[s, c] for s, c in dims])
    ap.offset = base_ap.offset + extra_offset
    return ap


def _switch8(tc, bid, hid):
    for b in range(4):
        with tc.If(bid == b):
            for half in range(2):
                with tc.If(hid == half):
                    yield 2 * b + half


def _hoist_extra_waits(nc: bass.Bass) -> None:
    """Split all but one sync-wait of any multi-wait instruction onto
    standalone EventSemaphore ops just before it (same in-order engine
    stream) -- walrus allows at most one wait per instruction."""
    for bb in nc.m.functions[0].blocks:
        insts = bb.instructions
        i = 0
        while i < len(insts):
            ins = insts[i]
            si = ins.sync_info
            if si is not None and len(si.on_wait) > 1 and ins.opcode != "EventSemaphore":
                waits = list(si.on_wait)
                for j, w in enumerate(waits[:-1]):
                    insts.insert(
                        i + j,
                        mybir.InstEventSemaphore(
                            name=f"{ins.name}_w{j}",
                            engine=ins.engine,
                            ins=[], outs=[],
                            sync_info=mybir.SyncInfo(on_wait=[w], on_update=[]),
                        ),
                    )
                si.on_wait = [waits[-1]]
                i += len(waits) - 1
            i += 1


def _layout_caps(lo, n):
    pc, ldm = 1, 2
    for b in range(B):
        for half in _split_assign(lo[b], n[b]):
            pc = max(pc, len(half))
        ldm = max(ldm, max(_ld_of(int(x)) for x in n[b][:, 0]))
    return pc, ldm


def _dims(dd):
    f = [d for d in dd if d[1] > 1]
    return f or [[1, 1]]


def _build_program(lo: np.ndarray, n: np.ndarray, hoist: bool = True) -> bass.Bass:
    nc = bass.Bass("TRN2", target_bir_lowering=False, debug=False, num_devices=8)
    PCAP, LDM = _layout_caps(lo, n)
    J_S, HB_S, WB_S = 4 * PCAP, 2 * PCAP, PCAP        # Bt strides
    # V/U layout is pi-major (pi:8, db:4, hb:2, wb:1) so stores read
    # contiguous runs (big DMA descriptors) while acc TTs keep a packed
    # innermost dim for the bf16 2x mode.

    fm_d = nc.dram_tensor("fm_part", [128, JS * S * S], BF, kind="ExternalInput")
    out_d = nc.dram_tensor("out_v", [64, 8 * PCAP], BF, kind="ExternalOutput")

    splits = [_split_assign(lo[b], n[b]) for b in range(B)]
    plans = [[_plan_core(lo[b], n[b], splits[b][h], PCAP) for h in (0, 1)]
             for b in range(B)]

    with tile.TileContext(nc) as tc:
        with tc.tile_pool(name="pool", bufs=1) as pool:
            A = pool.tile([128, JS * S * S], BF)
            Bt = pool.tile([128, JS * 4 * PCAP], BF)
            W = pool.tile([128, 4096], BF)
            U = [pool.tile([64, 8 * PCAP], BF, name=f"U{i}")
                 for i in range(LDM)]
            V = pool.tile([64, 8 * PCAP], BF)

            pid = nc.partition_id(
                engines=(mybir.EngineType.DVE, mybir.EngineType.Pool)
            )
            bid = pid >> 1
            hid = pid & 1

            qeng = {"SP": nc.sync, "Act": nc.scalar, "Pool": nc.gpsimd}
            for q, dmas in IN_PLAN.items():
                for (s0, cnt) in dmas:
                    src = fm_d.ap().copy()
                    src.ap = mybir.VecI64Pair(
                        [[JS * S * S, 128], [1, cnt * S * S]]
                    )
                    src.offset = s0 * S * S
                    qeng[q].dma_start(
                        _mk_ap(A[:], s0 * S * S, [[1, cnt * S * S]]), src)

            def emit_s1_tr(g, pi):
                nd, nh, nw, sd, sh, sw, lh, lw, dq0, ndq = g
                base_in = dq0 * (S * S) + sh * S + sw
                red = ([[S, lh]] if lh > 1 else []) + (
                    [[1, lw]] if lw > 1 else [])
                nred = len(red)
                ax = mybir.AxisListType.XY if nred == 2 else mybir.AxisListType.X
                if (1 if ndq > 1 else 0) + 2 + nred <= 4:
                    din = [[(nh // 2) * S, 2], [nw // 2, 2]]
                    dout = [[HB_S, 2], [WB_S, 2]]
                    if ndq > 1:
                        din.insert(0, [S * S, ndq])
                        dout.insert(0, [J_S, ndq])
                    nc.vector.tensor_reduce(
                        out=_mk_ap(Bt[:], pi, dout),
                        in_=_mk_ap(A[:], base_in, din + red),
                        axis=ax, op=mybir.AluOpType.max)
                else:
                    for hb in range(2):
                        nc.vector.tensor_reduce(
                            out=_mk_ap(Bt[:], pi + hb * HB_S,
                                       [[J_S, ndq], [WB_S, 2]]),
                            in_=_mk_ap(A[:], base_in + hb * (nh // 2) * S,
                                       [[S * S, ndq], [nw // 2, 2]] + red),
                            axis=ax, op=mybir.AluOpType.max)

            def emit_s1_foldy(g, pi):
                # even nw, lh > 1: one y-half fold (TT, 2x) per hb into W,
                # then one 4-dim TensorReduce per hb into Bt.
                nd, nh, nw, sd, sh, sw, lh, lw, dq0, ndq = g
                base_in = dq0 * (S * S) + sh * S + sw
                hy = (lh + 1) // 2
                WY, WJ, WHB = nw, nw * hy, nw * hy * ndq
                for hb in range(2):
                    o0 = base_in + hb * (nh // 2) * S
                    din = _dims([[S * S, ndq], [S, hy], [1, nw]])
                    nc.vector.tensor_tensor(
                        out=_mk_ap(W[:], hb * WHB,
                                   _dims([[WJ, ndq], [WY, hy], [1, nw]])),
                        in0=_mk_ap(A[:], o0, din),
                        in1=_mk_ap(A[:], o0 + (lh - hy) * S, din),
                        op=mybir.AluOpType.max)
                for hb in range(2):
                    nc.vector.tensor_reduce(
                        out=_mk_ap(Bt[:], pi + hb * HB_S,
                                   [[J_S, ndq], [WB_S, 2]]),
                        in_=_mk_ap(W[:], hb * WHB,
                                   [[WJ, ndq], [lw, 2], [WY, hy], [1, lw]]),
                        axis=mybir.AxisListType.XY, op=mybir.AluOpType.max)

            def emit_s1_foldx(g, pi):
                # even nh, lw > 1: one x-half fold (TT, 2x) per wb into W
                # (h-rows merged over both bins), then one TensorReduce per
                # hb into Bt.  W layout: (j', rows:nh, wb:2, x:hx).
                nd, nh, nw, sd, sh, sw, lh, lw, dq0, ndq = g
                base_in = dq0 * (S * S) + sh * S + sw
                hx = (lw + 1) // 2
                WR, WWBs, WJ = hx, 0, 0  # row stride hx, wb stride set below
                WWBs = nh * hx
                WJ = 2 * nh * hx
                for wb in range(2):
                    o0 = base_in + wb * (nw // 2)
                    din = _dims([[S * S, ndq], [S, nh], [1, hx]])
                    nc.vector.tensor_tensor(
                        out=_mk_ap(W[:], wb * WWBs,
                                   _dims([[WJ, ndq], [WR, nh], [1, hx]])),
                        in0=_mk_ap(A[:], o0, din),
                        in1=_mk_ap(A[:], o0 + (lw - hx), din),
                        op=mybir.AluOpType.max)
                for hb in range(2):
                    nc.vector.tensor_reduce(
                        out=_mk_ap(Bt[:], pi + hb * HB_S,
                                   [[J_S, ndq], [WB_S, 2]]),
                        in_=_mk_ap(W[:], hb * (nh // 2) * WR,
                                   [[WJ, ndq], [WWBs, 2], [WR, lh], [1, hx]]),
                        axis=mybir.AxisListType.XY, op=mybir.AluOpType.max)

            def emit_s1_cp(g, pi):
                nd, nh, nw, sd, sh, sw, lh, lw, dq0, ndq = g
                base_in = dq0 * (S * S) + sh * S + sw
                nc.gpsimd.tensor_copy(
                    _mk_ap(Bt[:], pi, _dims([[J_S, ndq], [HB_S, 2],
                                             [WB_S, 2]])),
                    _mk_ap(A[:], base_in,
                           _dims([[S * S, ndq], [(nh // 2) * S, 2],
                                  [nw // 2, 2]])))

            def v_slot(T, i0, m, db):
                # dims ordered (pi, hb, wb) to pair with bt_slot
                return _mk_ap(T[:], i0 * 8 + db * 4,
                              _dims([[8, m], [2, 2], [1, 2]]))

            def bt_slot(lane, j0, i0, m):
                return _mk_ap(Bt[lane * 64:(lane + 1) * 64], j0 * J_S + i0,
                              _dims([[1, m], [HB_S, 2], [WB_S, 2]]))

            for case in _switch8(tc, bid, hid):
                b, half = case // 2, case % 2
                plan = plans[b][half]
                lo_b, n_b = lo[b], n[b]
                geom = {p: _geom(lo_b, n_b, p) for p in plan["pi_order"]}
                gap0, gap1 = plan["n_front"], PCAP - plan["n_last"]
                if gap0 < gap1:
                    nc.gpsimd.memset(
                        _mk_ap(V[:], gap0 * 8, [[1, (gap1 - gap0) * 8]]), 0)
                for op in plan["ops"]:
                    k = op["kind"]
                    if k == "s1cp":
                        emit_s1_cp(geom[op["p"]], op["pi"])
                    elif k == "slice":
                        dst = V if op["k"] == 0 else U[op["k"]]
                        nc.gpsimd.tensor_copy(
                            v_slot(dst, op["i0"], op["m"], op["db"]),
                            bt_slot(op["lane"], op["j0"], op["i0"], op["m"]))
                    elif k == "s1":
                        sk = op["s1kind"]
                        if sk == "foldy":
                            emit_s1_foldy(geom[op["p"]], op["pi"])
                        elif sk == "foldx":
                            emit_s1_foldx(geom[op["p"]], op["pi"])
                        else:
                            emit_s1_tr(geom[op["p"]], op["pi"])
                    else:
                        g0, span = op["g0"], op["span"]
                        dims = _dims([[1, span * 8]])
                        vap = _mk_ap(V[:], g0 * 8, dims)
                        nc.vector.tensor_tensor(
                            out=vap, in0=vap,
                            in1=_mk_ap(U[op["k"]][:], g0 * 8, dims),
                            op=mybir.AluOpType.max)
            # Stores outside the per-core arms: conditional DMAs break the
            # tile framework's queue-sem watermark compensation.  The bulk
            # store excludes the TAILN tail columns, which hold the final
            # chunk's proposals and go out in a second, tiny store.
            nf = PCAP - TAILN
            nc.sync.dma_start(
                _mk_ap(out_d.ap(), 0, [[1, nf * 8]]),
                _mk_ap(V[:], 0, [[1, nf * 8]]))
            nc.scalar.dma_start(
                _mk_ap(out_d.ap(), nf * 8, [[1, TAILN * 8]]),
                _mk_ap(V[:], nf * 8, [[1, TAILN * 8]]))

    if hoist:
        _hoist_extra_waits(nc)
    return nc


def _prep_fm(fm_bf: np.ndarray):
    """fm[b] [C,24,24,24] bf16 -> [128=(dm2,c), 12*576] host pre-transpose."""
    x = fm_bf.reshape(C, JS, 2, S * S).transpose(2, 0, 1, 3)
    return np.ascontiguousarray(x.reshape(2 * C, JS * S * S))


def _unpack(res_k, lo_b, n_b, half):
    props = _split_assign(lo_b, n_b)[half]
    rv = np.asarray(res_k["out_v"]).astype(np.float32)
    PCAP = rv.shape[1] // 8
    cols = _plan_core(lo_b, n_b, props, PCAP)["columns"]
    rv = rv.reshape(C, PCAP, 2, 2, 2)      # [c, pi, db, hb, wb]
    return {p: rv[:, cols[p]] for p in props}


def _run(fm: np.ndarray, corners: np.ndarray, trace: bool = False, trace_cores=None):
    fm = np.asarray(fm, dtype=np.float32)
    corners = np.asarray(corners, dtype=np.float32)
    assert fm.shape == (B, C, S, S, S) and corners.shape == (B, P, 2, 3)

    lo, n = _bin_params(corners)
    nc = _build_program(lo, n)

    fm_bf = fm.astype(ml_dtypes.bfloat16)
    parts = [_prep_fm(fm_bf[b]) for b in range(B)]
    in_maps = [{"fm_part": parts[k // 2]} for k in range(8)]

    res = run_bass_kernel_spmd(
        nc, in_maps, core_ids=list(range(8)), trace=trace,
        **({"trace_cores": trace_cores} if trace_cores else {}),
    )

    out = np.empty((B, P, C, 2, 2, 2), dtype=np.float32)
    for k in range(8):
        b, half = k // 2, k % 2
        vals = _unpack(res.results[k], lo[b], n[b], half)
        for p, v in vals.items():
            out[b, p] = v
    return out, res


def kernel(fm: np.ndarray, corners: np.ndarray) -> np.ndarray:
    out, _ = _run(fm, corners)
    return out


# revision 49
# speedup vs baseline: 1.0072x; 1.0072x over previous
"""CropProposals v4: shuffle-free layered stage-2 + DVE fold-assisted stage-1.

Sharding: core k handles batch b = k//2 and a balanced half of its 64
proposals (~32), ALL 64 channels.  fm per core is pre-transposed on host to
[128 = (dm2:2, c:64), 12*576 = (dq2, h, w)] bf16 so multi-slab input DMAs
are 3-dim APs with large contiguous elements (9 DMAs over the SP/Act/Pool
queues, first slabs visible ~2.4us, all by ~3.9us).

Layouts (partition dim first):
  A  [128 = (dm2, c), free (dq2:12, h:24, w:24)]
  Bt [128 = (dm2, c), free (j':12, hb:2, wb:2, pi:PCAP)]  stage-1 results,
      j' normalized per proposal (j' = dq2 - sd//2)
  W  [128, scratch]   DVE fold scratch
  U[k] [64 = (c), free (pi:PCAP, db:2, hb:2, wb:2)]  d-slice layer k
  V  [64 = (c), same free layout]  final output -- pi-major, so the two
      stores read contiguous runs (large DMA descriptors)

Stage 1 (DVE): fused TensorReduce over (lh x lw) keeping (j', hb, wb).
For big crops with even nw (or nh) one half-fold tensor_tensor level (bf16
2x mode, 0.52 ns/elem vs TensorReduce's mode-less 1.04) runs first, then a
smaller TensorReduce.  1x1-bin crops are plain Pool copies.  Execution
order: input-arrival gate, then big-ld first.

Stage 2 (no DMA shuffle; walrus only allows tensor-tensor max on DVE, and
only with equal input partition bases): a d-bin of a proposal maps to one
d-slice per (bin, k) with lane dm2=(r+s2)%2, so Pool stages the k-th slice
of every (run, bin) into layer tile U[k] at partition base 0 (cross-base
1-input copies are legal); slice 0 goes straight into V.  pi columns are
sorted by ascending ld = (nd+1)//2 in three chunks [small-ld early |
big-ld | late small-ld (tail store region)], and ONE in-place DVE
tensor_tensor per level k accumulates U[k] into V over the contiguous
SUFFIX of proposals with ld > k -- one instruction per level for the whole
core amortizes DVE's ~60ns fixed cost, and high levels touch only the
trailing columns.  The output leaves in a bulk store plus a tiny tail
store covering the last TAILN columns (the late chunk).

The host-side planner (_plan_core) models engine clocks and emits all ops
in modeled start order; the tile framework infers exact semaphores from
program order.  DMAs stay outside the per-core branch arms (conditional
DMAs break walrus queue-sem watermarks).

bf16 end-to-end: max() commutes with monotone rounding, so the result
equals round_bf16(exact f32 result); rel err <= 2^-8 ~ 4e-3 within the
2e-2 gate.
"""

import os
import sys

import numpy as np

for _p in ("/opt/trn_rl_repo", os.path.expanduser("~/.axon_site/_ro/trn_rl_repo")):
    if os.path.isdir(_p) and _p not in sys.path:
        sys.path.insert(0, _p)

import ml_dtypes  # noqa: E402

import concourse.bass as bass  # noqa: E402
import concourse.tile as tile  # noqa: E402
import concourse.tile_sem_assignment as _tsa  # noqa: E402
from concourse import mybir  # noqa: E402
from concourse.bass_utils import run_bass_kernel_spmd  # noqa: E402

# Walrus encodes at most ONE sync-wait per compute instruction.  Pin HWDGE
# completion-sem lanes by issuing engine so every consumer waits on one
# per-queue FIFO sem; extra waits are split off by _hoist_extra_waits.
if not getattr(_tsa, "_ant_engine_keyed_lanes", False):
    _orig_assign_tick = _tsa.TileClockTick._assign_tick

    def _assign_tick_engine_lanes(self, inst):
        if isinstance(inst, _tsa.DMAInst) and inst.engine in (
            mybir.EngineType.SP,
            mybir.EngineType.Activation,
        ):
            self.next_hw_dma_idx = 0 if inst.engine == mybir.EngineType.SP else 1
        return _orig_assign_tick(self, inst)

    _tsa.TileClockTick._assign_tick = _assign_tick_engine_lanes
    _tsa._ant_engine_keyed_lanes = True

B, P, C, S = 4, 64, 64, 24
JS = 12
BF = mybir.dt.bfloat16

# cost-model constants (ns)
DVE_EL = 1.0417
DVE_FIX = 60.4
DVE_EL2 = 0.5208     # bf16 packed tensor_tensor (2x mode)
POOL_CP = 0.45       # Pool tensor_copy ns/elem (observed rate)
POOL_OP = 3.0
SEM_HOP = 100.0
CHUNK_CUTS = (500.0,)  # B/C boundary, ns before the last s1 finish
TAILN = 8            # pi columns in the tiny trailing store
LD_A = 4             # ld >= LD_A proposals form the early long-chain chunk

# input DMA plan: per queue, list of (first_slab, n_slabs)
IN_PLAN = {
    "SP": [(0, 1), (3, 2), (9, 1)],
    "Act": [(1, 1), (5, 2), (10, 1)],
    "Pool": [(2, 1), (7, 2), (11, 1)],
}


def _bin_params(corners: np.ndarray):
    """Replicate the reference's float32 bin math. Returns lo, n int64 [B,P,3]
    with axis order (d, h, w)."""
    c = corners.astype(np.float32) / np.float32(4.0)
    LL = np.clip(c[:, :, 0, :], np.float32(0.0), np.float32(21.0))
    UR = c[:, :, 1, :]
    UR = np.where(UR - LL >= np.float32(2.0), UR, LL + np.float32(2.0))
    UR = np.clip(UR, np.float32(2.0), np.float32(23.0))
    lo = np.floor(LL).astype(np.int64)
    n = np.floor(UR).astype(np.int64) - lo
    return lo, n


def _geom(lo_b, n_b, p):
    nd, nh, nw = (int(n_b[p, a]) for a in range(3))
    sd, sh, sw = (int(lo_b[p, a]) for a in range(3))
    lh, lw = (nh + 1) // 2, (nw + 1) // 2
    dq0 = sd // 2
    ndq = (sd + nd - 1) // 2 - dq0 + 1
    return nd, nh, nw, sd, sh, sw, lh, lw, dq0, ndq


def _dma_cost(n_slabs):
    tr = 128 / 16.0 * (n_slabs * 1152 / 22.5)
    return max(500.0, tr + 70.0)


def _slab_vis():
    delays = {"SP": 1717.0, "Act": 1717.0, "Pool": 1883.0}
    t0 = {"SP": 200.0, "Act": 200.0, "Pool": 100.0}
    vis = {}
    for q, dmas in IN_PLAN.items():
        t = t0[q]
        for (s0, cnt) in dmas:
            t += _dma_cost(cnt)
            for s in range(s0, s0 + cnt):
                vis[s] = t + delays[q]
    assert sorted(vis) == list(range(JS)), vis
    return vis


def _pool_busy_until():
    return 100.0 + sum(_dma_cost(c) for _, c in IN_PLAN["Pool"])


def _s1_kind(g):
    """Returns (kind, modeled ns).  kind: 'cp' (Pool), 'tr', 'foldy', 'foldx'."""
    nd, nh, nw, sd, sh, sw, lh, lw, dq0, ndq = g
    K = ndq * 4 * lh * lw
    if lh == 1 and lw == 1:
        return "cp", max(POOL_OP, ndq * 4 * POOL_CP)
    ni = 2 if (ndq > 1 and lh > 1 and lw > 1) else 1
    best = ("tr", ni * DVE_FIX + K * DVE_EL)
    if nw % 2 == 0 and lh > 1 and lw > 1:
        hy = (lh + 1) // 2
        c = 2 * (DVE_FIX + ndq * hy * nw * DVE_EL2) \
            + 2 * (DVE_FIX + ndq * 2 * hy * lw * DVE_EL)
        if c < best[1]:
            best = ("foldy", c)
    if nh % 2 == 0 and lh > 1 and lw > 1:
        hx = (lw + 1) // 2
        c = 2 * (DVE_FIX + ndq * nh * hx * DVE_EL2) \
            + 2 * (DVE_FIX + ndq * 2 * lh * hx * DVE_EL)
        if c < best[1]:
            best = ("foldx", c)
    return best


def _ld_of(nd):
    return (nd + 1) // 2


def _lane_slots(nd, s2, db):
    """[(lane, j0, nj)] with nj>0 for bin db of a (nd, s2) class."""
    ld = _ld_of(nd)
    a = db * (nd // 2)
    b = a + ld
    out = []
    for l in (0, 1):
        r0 = a + ((l - s2 - a) % 2)
        if r0 < b:
            nj = (b - r0 + 1) // 2
            out.append((l, (r0 - l + s2) // 2, nj))
    return out


def _slice_of(nd, s2, db, k):
    """(lane, j') of the k-th d-slice of bin db."""
    r = db * (nd // 2) + k
    l = (r + s2) % 2
    return l, (r - l + s2) // 2


def _pi_order(lo_b, n_b, props, chunk_of=None):
    # ascending ld within a chunk: accumulation level k covers the SUFFIX
    # of proposals with ld > k, so each chunk's last-written columns sit at
    # its end (next to the store-tail region).
    key = lambda p: ((chunk_of[p] if chunk_of else 0),
                     _ld_of(int(n_b[p, 0])), int(n_b[p, 0]),
                     int(lo_b[p, 0]) % 2, int(lo_b[p, 0]), p)
    return sorted(props, key=key)


GB_MIN_NET = 100.0


def _pick_families(geom, kind):
    """Batch thin crops (lh==1 or lw==1) of one core into padded family
    cells so ONE TensorReduce per family replaces per-proposal ones."""
    fams = []
    for fam, sel in (("x", lambda g: g[6] == 1 and g[7] > 1),
                     ("y", lambda g: g[7] == 1 and g[6] > 1)):
        cands = []
        for p, g in geom.items():
            if kind[p][0] != "tr" or not sel(g):
                continue
            ln = g[6] if fam == "y" else g[7]   # lh or lw
            cands.append((p, ln, g[9]))         # (p, len, ndq)
        if len(cands) < 3:
            continue
        best = None
        for w in sorted({ln for _, ln, _ in cands}):
            mem = [(p, ln, ndq) for p, ln, ndq in cands
                   if ln <= w and DVE_FIX - (w - ln) * ndq * 4 * DVE_EL > 15.0]
            if len(mem) < 3:
                continue
            net = sum(DVE_FIX - (w - ln) * ndq * 4 * DVE_EL
                      for _, ln, ndq in mem) - DVE_FIX
            if best is None or net > best[0]:
                best = (net, w, mem)
        if best is None or best[0] < GB_MIN_NET:
            continue
        net, w, mem = best
        slot0, members = 0, []
        for p, ln, ndq in sorted(mem):
            members.append((p, slot0))
            slot0 += ndq
        fams.append(dict(fam=fam, w=w, members=members, ns=slot0))
    return fams


_split_cache = {}


def _split_assign(lo_b, n_b):
    """Split 64 proposals into two halves minimizing the max modeled end."""
    key = (lo_b.tobytes(), n_b.tobytes())
    if key in _split_cache:
        return _split_cache[key]
    costs = [(_s1_kind(_geom(lo_b, n_b, p))[1], p) for p in range(P)]
    costs.sort(key=lambda t: (-t[0], t[1]))
    halves, tot = ([], []), [0.0, 0.0]
    for c, p in costs:
        h = 0 if tot[0] <= tot[1] else 1
        halves[h].append(p)
        tot[h] += c

    def end(props):
        return _plan_core(lo_b, n_b, props, P)["model_end"]

    e = [end(halves[0]), end(halves[1])]
    for _ in range(8):
        d = 0 if e[0] >= e[1] else 1
        best = (max(e), None)
        for p in halves[d]:
            h0 = [x for x in halves[d] if x != p]
            h1 = halves[1 - d] + [p]
            ne = (end(h0), end(h1))
            if max(ne) < best[0] - 1.0:
                best = (max(ne), (p, ne))
        if best[1] is None:
            break
        p, ne = best[1]
        halves[d].remove(p)
        halves[1 - d].append(p)
        e = [ne[0], ne[1]] if d == 0 else [ne[1], ne[0]]
    res = (sorted(halves[0]), sorted(halves[1]))
    _split_cache[key] = res
    return res


def _runs_of(props, lo_b, n_b):
    i = 0
    while i < len(props):
        nd = int(n_b[props[i], 0])
        s2 = int(lo_b[props[i], 0]) % 2
        m = 1
        while (i + m < len(props)
               and int(n_b[props[i + m], 0]) == nd
               and int(lo_b[props[i + m], 0]) % 2 == s2):
            m += 1
        yield i, m, nd, s2
        i += m


def _plan_core(lo_b, n_b, props, pcap):
    """Schedule one core.  Returns pi columns and one merged ordered op list."""
    vis = _slab_vis()
    geom = {p: _geom(lo_b, n_b, p) for p in props}

    def gate(p):
        g = geom[p]
        dq0, ndq = g[8], g[9]
        return max(vis[q] for q in range(dq0, dq0 + ndq))

    kind = {p: _s1_kind(geom[p]) for p in props}
    fams = _pick_families(geom, kind)
    gb = {}           # p -> (fam_idx, slot0)
    for fi, f in enumerate(fams):
        for p, s0 in f["members"]:
            gb[p] = (fi, s0)
    goff, g2off = [], []
    gsz = g2sz = 0
    for f in fams:
        goff.append(gsz)
        g2off.append(g2sz)
        gsz += f["ns"] * 4 * f["w"]
        g2sz += f["ns"] * 4

    TD, TP = 0.0, _pool_busy_until()
    s1_fin = {}
    s1_sched = []
    gb_ops = []       # (start, op) for Pool/DVE family ops
    if fams:
        c = max(POOL_OP, gsz * 0.833)
        gb_ops.append((TP, dict(kind="gmem", size=gsz)))
        TP += c
        frel = [0.0] * len(fams)
        gmem_done = TP
        for p in sorted(gb, key=gate):
            fi, s0 = gb[p]
            cg = max(POOL_OP, geom[p][9] * 4 * (geom[p][6] * geom[p][7]) * POOL_CP)
            for hb in range(2):
                st = max(TP, gate(p), gmem_done)
                TP = st + cg / 2
                gb_ops.append((st, dict(kind="gath", p=p, hb=hb)))
            frel[fi] = max(frel[fi], TP + SEM_HOP)

    # stage-1 priority: arrival gate, then big-ld first (so the trailing
    # accumulation chains belong to small-ld chunks), then big cost first
    tasks = [(gate(p), -_ld_of(int(n_b[p, 0])), -kind[p][1], "p", p)
             for p in props if p not in gb]
    for fi, f in enumerate(fams):
        cost = DVE_FIX + f["ns"] * 4 * f["w"] * DVE_EL
        tasks.append((frel[fi], 0, -cost, "f", fi))
    tasks.sort()
    for g0k, _, negc, tk, x in tasks:
        if tk == "f":
            f = fams[x]
            st = max(TD, g0k)
            TD = st - negc if False else st + (DVE_FIX + f["ns"] * 4 * f["w"] * DVE_EL)
            gb_ops.append((st, dict(kind="ctr", fi=x)))
            for p, s0 in f["members"]:
                cb = max(POOL_OP, geom[p][9] * 8 * POOL_CP)
                stc = max(TP, TD + SEM_HOP)
                TP = stc + cb
                gb_ops.append((stc, dict(kind="cback", p=p)))
                s1_fin[p] = TP
            continue
        p = x
        k, c = kind[p]
        if k == "cp":
            st = max(TP, g0k)
            TP = st + c
            s1_fin[p] = TP
        else:
            st = max(TD, g0k)
            TD = st + c
            s1_fin[p] = TD
        s1_sched.append((st, p, k))

    # chunks: 0 = small-ld early, 1 = big-ld (placed after so ld ascends
    # over chunks 0+1 and ONE merged suffix-level chain serves both; its
    # high levels touch only the trailing columns next to the tail store),
    # 2 = last-finishing small-ld proposals, capped to the tail region.
    fmax = max(s1_fin.values())
    cut = CHUNK_CUTS[-1]

    def _chunk(p):
        if _ld_of(int(n_b[p, 0])) >= LD_A:
            return 1
        return 2 if s1_fin[p] > fmax - cut else 0
    chunk_of = {p: _chunk(p) for p in props}
    last = sorted([p for p in props if chunk_of[p] == 2],
                  key=lambda p: s1_fin[p])
    for p in last[:-TAILN] if len(last) > TAILN else []:
        chunk_of[p] = 0
    ncut = 2
    if all(v == ncut for v in chunk_of.values()):
        chunk_of = {p: 0 for p in props}
    pi_order = _pi_order(lo_b, n_b, props, chunk_of)
    # pi columns: front chunks pack from 0; the last chunk is pinned to the
    # final TAILN-capped columns so the split store's tail is uniform.
    n_last = sum(1 for p in props if chunk_of[p] == ncut)
    n_front = len(props) - n_last
    col = {}
    for i, p in enumerate(pi_order):
        col[p] = i if i < n_front else pcap - n_last + (i - n_front)
    pi_of = col

    dve_ops = []
    pool_ops = []
    for st, p, k in s1_sched:
        if k == "cp":
            pool_ops.append((st, dict(kind="s1cp", p=p, pi=pi_of[p])))
        else:
            dve_ops.append((st, dict(kind="s1", p=p, pi=pi_of[p], s1kind=k)))

    # ---- stage-2 ----
    rinfo = []
    i = 0
    while i < len(pi_order):
        nd = int(n_b[pi_order[i], 0])
        s2 = int(lo_b[pi_order[i], 0]) % 2
        m = 1
        while (i + m < len(pi_order)
               and int(n_b[pi_order[i + m], 0]) == nd
               and int(lo_b[pi_order[i + m], 0]) % 2 == s2
               and col[pi_order[i + m]] == col[pi_order[i]] + m):
            m += 1
        members = pi_order[i:i + m]
        hop = SEM_HOP if any(kind[p][0] != "cp" for p in members) else 0.0
        ready = max(s1_fin[p] for p in members) + hop
        rinfo.append(dict(i0=col[pi_order[i]], m=m, nd=nd, s2=s2,
                          ld=_ld_of(nd), ready=ready,
                          chunk=chunk_of[pi_order[i]]))
        i += m

    # Pool slice copies: k-th d-slice of each (run, bin); k==0 goes straight
    # to V, k>=1 to layer tile U[k].  Track per-(chunk, k) completion.
    layer_done = {}
    for r in sorted(rinfo, key=lambda r: r["ready"]):
        i0, m, nd, s2, ld = r["i0"], r["m"], r["nd"], r["s2"], r["ld"]
        c1 = max(POOL_OP, m * 4 * POOL_CP)
        for k in range(ld):
            for db in range(2):
                l, j = _slice_of(nd, s2, db, k)
                st = max(TP, r["ready"])
                TP = st + c1
                pool_ops.append((st, dict(kind="slice", i0=i0, m=m, db=db,
                                          k=k, lane=l, j0=j)))
            key = (r["chunk"], k)
            layer_done[key] = max(layer_done.get(key, 0.0), TP)

    # DVE level accumulations: one merged chain over chunks 0+1 (ld ascends
    # across both), a separate short chain for the tail chunk 2.  Level k
    # covers the SUFFIX of proposals with ld > k.
    for cids in ((0, 1), (2,)):
        cprops = [p for p in pi_order if chunk_of[p] in cids]
        if not cprops:
            continue
        cend = max(col[p] for p in cprops) + 1
        lds = [_ld_of(int(n_b[p, 0])) for p in cprops]
        ldmax = max(lds)
        prev = 0.0
        for k in range(1, ldmax):
            span = sum(1 for x in lds if x > k)
            rel = prev
            for cid in cids:
                rel = max(rel, layer_done.get((cid, k), 0.0) + SEM_HOP,
                          layer_done.get((cid, 0), 0.0) + SEM_HOP)
            st = max(TD, rel)
            TD = st + DVE_FIX + span * 8 * DVE_EL2
            prev = TD
            dve_ops.append((st, dict(kind="acc", g0=cend - span, span=span,
                                     k=k)))
    model_end = max(TD, TP)
    # Merge to one stream ordered by modeled start: tile deps are inferred
    # from program order, so every consumer must be emitted after its
    # producer (consumer start >= producer finish guarantees this).
    s1k = ("s1", "s1cp", "gmem", "gath", "ctr", "cback")
    allops = [(st, 0 if o["kind"] in s1k else 1, i, o)
              for i, (st, o) in enumerate(dve_ops + pool_ops + gb_ops)]
    allops.sort(key=lambda t: (t[0], t[1], t[2]))
    return dict(columns={p: col[p] for p in props}, pi_order=pi_order,
                ops=[o for _, _, _, o in allops],
                n_front=n_front, n_last=n_last, fams=fams, gb=gb,
                goff=goff, g2off=g2off, gsz=gsz, g2sz=g2sz,
                nprops=len(props), model_end=model_end)


def _mk_ap(base_ap, extra_offset: int, dims):
    ap = base_ap.copy()
    ap.ap = mybir.VecI64Pair([list(base_ap.ap[0])] + [[s, c] for s, c in dims])
    ap.offset = base_ap.offset + extra_offset
    return ap


def _switch8(tc, bid, hid):
    for b in range(4):
        with tc.If(bid == b):
            for half in range(2):
                with tc.If(hid == half):
                    yield 2 * b + half


def _hoist_extra_waits(nc: bass.Bass) -> None:
    """Split all but one sync-wait of any multi-wait instruction onto
    standalone EventSemaphore ops just before it (same in-order engine
    stream) -- walrus allows at most one wait per instruction."""
    for bb in nc.m.functions[0].blocks:
        insts = bb.instructions
        i = 0
        while i < len(insts):
            ins = insts[i]
            si = ins.sync_info
            if si is not None and len(si.on_wait) > 1 and ins.opcode != "EventSemaphore":
                waits = list(si.on_wait)
                for j, w in enumerate(waits[:-1]):
                    insts.insert(
                        i + j,
                        mybir.InstEventSemaphore(
                            name=f"{ins.name}_w{j}",
                            engine=ins.engine,
                            ins=[], outs=[],
                            sync_info=mybir.SyncInfo(on_wait=[w], on_update=[]),
                        ),
                    )
                si.on_wait = [waits[-1]]
                i += len(waits) - 1
            i += 1


def _layout_caps(lo, n):
    pc, ldm = 1, 2
    for b in range(B):
        for half in _split_assign(lo[b], n[b]):
            pc = max(pc, len(half))
        ldm = max(ldm, max(_ld_of(int(x)) for x in n[b][:, 0]))
    return pc, ldm


def _dims(dd):
    f = [d for d in dd if d[1] > 1]
    return f or [[1, 1]]


def _build_program(lo: np.ndarray, n: np.ndarray, hoist: bool = True) -> bass.Bass:
    nc = bass.Bass("TRN2", target_bir_lowering=False, debug=False, num_devices=8)
    PCAP, LDM = _layout_caps(lo, n)
    J_S, HB_S, WB_S = 4 * PCAP, 2 * PCAP, PCAP        # Bt strides
    # V/U layout is pi-major (pi:8, db:4, hb:2, wb:1) so stores read
    # contiguous runs (big DMA descriptors) while acc TTs keep a packed
    # innermost dim for the bf16 2x mode.

    fm_d = nc.dram_tensor("fm_part", [128, JS * S * S], BF, kind="ExternalInput")
    out_d = nc.dram_tensor("out_v", [64, 8 * PCAP], BF, kind="ExternalOutput")

    splits = [_split_assign(lo[b], n[b]) for b in range(B)]
    plans = [[_plan_core(lo[b], n[b], splits[b][h], PCAP) for h in (0, 1)]
             for b in range(B)]

    with tile.TileContext(nc) as tc:
        with tc.tile_pool(name="pool", bufs=1) as pool:
            A = pool.tile([128, JS * S * S], BF)
            Bt = pool.tile([128, JS * 4 * PCAP], BF)
            W = pool.tile([128, 4096], BF)
            U = [pool.tile([64, 8 * PCAP], BF, name=f"U{i}")
                 for i in range(LDM)]
            V = pool.tile([64, 8 * PCAP], BF)
            GCAP = max([1] + [pl["gsz"] for bb_ in plans for pl in bb_])
            G2CAP = max([1] + [pl["g2sz"] for bb_ in plans for pl in bb_])
            G = pool.tile([128, GCAP], BF)
            G2 = pool.tile([128, G2CAP], BF)

            pid = nc.partition_id(
                engines=(mybir.EngineType.DVE, mybir.EngineType.Pool)
            )
            bid = pid >> 1
            hid = pid & 1

            qeng = {"SP": nc.sync, "Act": nc.scalar, "Pool": nc.gpsimd}
            for q, dmas in IN_PLAN.items():
                for (s0, cnt) in dmas:
                    src = fm_d.ap().copy()
                    src.ap = mybir.VecI64Pair(
                        [[JS * S * S, 128], [1, cnt * S * S]]
                    )
                    src.offset = s0 * S * S
                    qeng[q].dma_start(
                        _mk_ap(A[:], s0 * S * S, [[1, cnt * S * S]]), src)

            def emit_s1_tr(g, pi):
                nd, nh, nw, sd, sh, sw, lh, lw, dq0, ndq = g
                base_in = dq0 * (S * S) + sh * S + sw
                red = ([[S, lh]] if lh > 1 else []) + (
                    [[1, lw]] if lw > 1 else [])
                nred = len(red)
                ax = mybir.AxisListType.XY if nred == 2 else mybir.AxisListType.X
                if (1 if ndq > 1 else 0) + 2 + nred <= 4:
                    din = [[(nh // 2) * S, 2], [nw // 2, 2]]
                    dout = [[HB_S, 2], [WB_S, 2]]
                    if ndq > 1:
                        din.insert(0, [S * S, ndq])
                        dout.insert(0, [J_S, ndq])
                    nc.vector.tensor_reduce(
                        out=_mk_ap(Bt[:], pi, dout),
                        in_=_mk_ap(A[:], base_in, din + red),
                        axis=ax, op=mybir.AluOpType.max)
                else:
                    for hb in range(2):
                        nc.vector.tensor_reduce(
                            out=_mk_ap(Bt[:], pi + hb * HB_S,
                                       [[J_S, ndq], [WB_S, 2]]),
                            in_=_mk_ap(A[:], base_in + hb * (nh // 2) * S,
                                       [[S * S, ndq], [nw // 2, 2]] + red),
                            axis=ax, op=mybir.AluOpType.max)

            def emit_s1_foldy(g, pi):
                # even nw, lh > 1: one y-half fold (TT, 2x) per hb into W,
                # then one 4-dim TensorReduce per hb into Bt.
                nd, nh, nw, sd, sh, sw, lh, lw, dq0, ndq = g
                base_in = dq0 * (S * S) + sh * S + sw
                hy = (lh + 1) // 2
                WY, WJ, WHB = nw, nw * hy, nw * hy * ndq
                for hb in range(2):
                    o0 = base_in + hb * (nh // 2) * S
                    din = _dims([[S * S, ndq], [S, hy], [1, nw]])
                    nc.vector.tensor_tensor(
                        out=_mk_ap(W[:], hb * WHB,
                                   _dims([[WJ, ndq], [WY, hy], [1, nw]])),
                        in0=_mk_ap(A[:], o0, din),
                        in1=_mk_ap(A[:], o0 + (lh - hy) * S, din),
                        op=mybir.AluOpType.max)
                for hb in range(2):
                    nc.vector.tensor_reduce(
                        out=_mk_ap(Bt[:], pi + hb * HB_S,
                                   [[J_S, ndq], [WB_S, 2]]),
                        in_=_mk_ap(W[:], hb * WHB,
                                   [[WJ, ndq], [lw, 2], [WY, hy], [1, lw]]),
                        axis=mybir.AxisListType.XY, op=mybir.AluOpType.max)

            def emit_s1_foldx(g, pi):
                # even nh, lw > 1: one x-half fold (TT, 2x) per wb into W
                # (h-rows merged over both bins), then one TensorReduce per
                # hb into Bt.  W layout: (j', rows:nh, wb:2, x:hx).
                nd, nh, nw, sd, sh, sw, lh, lw, dq0, ndq = g
                base_in = dq0 * (S * S) + sh * S + sw
                hx = (lw + 1) // 2
                WR, WWBs, WJ = hx, 0, 0  # row stride hx, wb stride set below
                WWBs = nh * hx
                WJ = 2 * nh * hx
                for wb in range(2):
                    o0 = base_in + wb * (nw // 2)
                    din = _dims([[S * S, ndq], [S, nh], [1, hx]])
                    nc.vector.tensor_tensor(
                        out=_mk_ap(W[:], wb * WWBs,
                                   _dims([[WJ, ndq], [WR, nh], [1, hx]])),
                        in0=_mk_ap(A[:], o0, din),
                        in1=_mk_ap(A[:], o0 + (lw - hx), din),
                        op=mybir.AluOpType.max)
                for hb in range(2):
                    nc.vector.tensor_reduce(
                        out=_mk_ap(Bt[:], pi + hb * HB_S,
                                   [[J_S, ndq], [WB_S, 2]]),
                        in_=_mk_ap(W[:], hb * (nh // 2) * WR,
                                   [[WJ, ndq], [WWBs, 2], [WR, lh], [1, hx]]),
                        axis=mybir.AxisListType.XY, op=mybir.AluOpType.max)

            def emit_s1_cp(g, pi):
                nd, nh, nw, sd, sh, sw, lh, lw, dq0, ndq = g
                base_in = dq0 * (S * S) + sh * S + sw
                nc.gpsimd.tensor_copy(
                    _mk_ap(Bt[:], pi, _dims([[J_S, ndq], [HB_S, 2],
                                             [WB_S, 2]])),
                    _mk_ap(A[:], base_in,
                           _dims([[S * S, ndq], [(nh // 2) * S, 2],
                                  [nw // 2, 2]])))

            def v_slot(T, i0, m, db):
                # dims ordered (pi, hb, wb) to pair with bt_slot
                return _mk_ap(T[:], i0 * 8 + db * 4,
                              _dims([[8, m], [2, 2], [1, 2]]))

            def bt_slot(lane, j0, i0, m):
                return _mk_ap(Bt[lane * 64:(lane + 1) * 64], j0 * J_S + i0,
                              _dims([[1, m], [HB_S, 2], [WB_S, 2]]))

            for case in _switch8(tc, bid, hid):
                b, half = case // 2, case % 2
                plan = plans[b][half]
                lo_b, n_b = lo[b], n[b]
                geom = {p: _geom(lo_b, n_b, p) for p in plan["pi_order"]}
                gap0, gap1 = plan["n_front"], PCAP - plan["n_last"]
                if gap0 < gap1:
                    nc.gpsimd.memset(
                        _mk_ap(V[:], gap0 * 8, [[1, (gap1 - gap0) * 8]]), 0)
                for op in plan["ops"]:
                    k = op["kind"]
                    if k == "gmem":
                        nc.gpsimd.memset(
                            _mk_ap(G[:], 0, [[1, plan["gsz"]]]), -3.0e38)
                    elif k == "gath":
                        p, hb = op["p"], op["hb"]
                        fi, s0 = plan["gb"][p]
                        f = plan["fams"][fi]
                        nd, nh, nw, sd, sh, sw, lh, lw, dq0, ndq = geom[p]
                        base_in = dq0 * S * S + sh * S + sw + hb * (nh // 2) * S
                        w = f["w"]
                        base_g = plan["goff"][fi] + s0 * 4 * w + hb * 2 * w
                        if f["fam"] == "x":
                            out = _mk_ap(G[:], base_g,
                                         _dims([[4 * w, ndq], [w, 2], [1, lw]]))
                            din = _mk_ap(A[:], base_in,
                                         _dims([[S * S, ndq], [nw // 2, 2],
                                                [1, lw]]))
                        else:
                            out = _mk_ap(G[:], base_g,
                                         _dims([[4 * w, ndq], [2, lh], [1, 2]]))
                            din = _mk_ap(A[:], base_in,
                                         _dims([[S * S, ndq], [S, lh],
                                                [nw // 2, 2]]))
                        nc.gpsimd.tensor_copy(out, din)
                    elif k == "ctr":
                        f = plan["fams"][op["fi"]]
                        w, ns = f["w"], f["ns"]
                        go, g2o = plan["goff"][op["fi"]], plan["g2off"][op["fi"]]
                        if f["fam"] == "x":
                            nc.vector.tensor_reduce(
                                out=_mk_ap(G2[:], g2o, [[1, ns * 4]]),
                                in_=_mk_ap(G[:], go, [[w, ns * 4], [1, w]]),
                                axis=mybir.AxisListType.X,
                                op=mybir.AluOpType.max)
                        else:
                            nc.vector.tensor_reduce(
                                out=_mk_ap(G2[:], g2o, [[2, ns * 2], [1, 2]]),
                                in_=_mk_ap(G[:], go,
                                           [[2 * w, ns * 2], [1, 2], [2, w]]),
                                axis=mybir.AxisListType.X,
                                op=mybir.AluOpType.max)
                    elif k == "cback":
                        p = op["p"]
                        fi, s0 = plan["gb"][p]
                        ndq = geom[p][9]
                        pi = plan["columns"][p]
                        nc.gpsimd.tensor_copy(
                            _mk_ap(Bt[:], pi, _dims([[J_S, ndq], [HB_S, 2],
                                                     [WB_S, 2]])),
                            _mk_ap(G2[:], plan["g2off"][fi] + s0 * 4,
                                   _dims([[4, ndq], [2, 2], [1, 2]])))
                    elif k == "s1cp":
                        emit_s1_cp(geom[op["p"]], op["pi"])
                    elif k == "slice":
                        dst = V if op["k"] == 0 else U[op["k"]]
                        nc.gpsimd.tensor_copy(
                            v_slot(dst, op["i0"], op["m"], op["db"]),
                            bt_slot(op["lane"], op["j0"], op["i0"], op["m"]))
                    elif k == "s1":
                        sk = op["s1kind"]
                        if sk == "foldy":
                            emit_s1_foldy(geom[op["p"]], op["pi"])
                        elif sk == "foldx":
                            emit_s1_foldx(geom[op["p"]], op["pi"])
                        else:
                            emit_s1_tr(geom[op["p"]], op["pi"])
                    else:
                        g0, span = op["g0"], op["span"]
                        dims = _dims([[1, span * 8]])
                        vap = _mk_ap(V[:], g0 * 8, dims)
                        nc.vector.tensor_tensor(
                            out=vap, in0=vap,
                            in1=_mk_ap(U[op["k"]][:], g0 * 8, dims),
                            op=mybir.AluOpType.max)
            # Stores outside the per-core arms: conditional DMAs break the
            # tile framework's queue-sem watermark compensation.  The bulk
            # store excludes the TAILN tail columns, which hold the final
            # chunk's proposals and go out in a second, tiny store.
            nf = PCAP - TAILN
            nc.sync.dma_start(
                _mk_ap(out_d.ap(), 0, [[1, nf * 8]]),
                _mk_ap(V[:], 0, [[1, nf * 8]]))
            nc.scalar.dma_start(
                _mk_ap(out_d.ap(), nf * 8, [[1, TAILN * 8]]),
                _mk_ap(V[:], nf * 8, [[1, TAILN * 8]]))

    if hoist:
        _hoist_extra_waits(nc)
    return nc


def _prep_fm(fm_bf: np.ndarray):
    """fm[b] [C,24,24,24] bf16 -> [128=(dm2,c), 12*576] host pre-transpose."""
    x = fm_bf.reshape(C, JS, 2, S * S).transpose(2, 0, 1, 3)
    return np.ascontiguousarray(x.reshape(2 * C, JS * S * S))


def _unpack(res_k, lo_b, n_b, half):
    props = _split_assign(lo_b, n_b)[half]
    rv = np.asarray(res_k["out_v"]).astype(np.float32)
    PCAP = rv.shape[1] // 8
    cols = _plan_core(lo_b, n_b, props, PCAP)["columns"]
    rv = rv.reshape(C, PCAP, 2, 2, 2)      # [c, pi, db, hb, wb]
    return {p: rv[:, cols[p]] for p in props}


def _run(fm: np.ndarray, corners: np.ndarray, trace: bool = False, trace_cores=None):
    fm = np.asarray(fm, dtype=np.float32)
    corners = np.asarray(corners, dtype=np.float32)
    assert fm.shape == (B, C, S, S, S) and corners.shape == (B, P, 2, 3)

    lo, n = _bin_params(corners)
    nc = _build_program(lo, n)

    fm_bf = fm.astype(ml_dtypes.bfloat16)
    parts = [_prep_fm(fm_bf[b]) for b in range(B)]
    in_maps = [{"fm_part": parts[k // 2]} for k in range(8)]

    res = run_bass_kernel_spmd(
        nc, in_maps, core_ids=list(range(8)), trace=trace,
        **({"trace_cores": trace_cores} if trace_cores else {}),
    )

    out = np.empty((B, P, C, 2, 2, 2), dtype=np.float32)
    for k in range(8):
        b, half = k // 2, k % 2
        vals = _unpack(res.results[k], lo[b], n[b], half)
        for p, v in vals.items():
            out[b, p] = v
    return out, res


def kernel(fm: np.ndarray, corners: np.ndarray) -> np.ndarray:
    out, _ = _run(fm, corners)
    return out


# revision 52
# speedup vs baseline: 1.0113x; 1.0041x over previous
"""CropProposals v4: shuffle-free layered stage-2 + DVE fold-assisted stage-1.

Sharding: core k handles batch b = k//2 and a balanced half of its 64
proposals (~32), ALL 64 channels.  fm per core is pre-transposed on host to
[128 = (dm2:2, c:64), 12*576 = (dq2, h, w)] bf16 so multi-slab input DMAs
are 3-dim APs with large contiguous elements (9 DMAs over the SP/Act/Pool
queues, first slabs visible ~2.4us, all by ~3.9us).

Layouts (partition dim first):
  A  [128 = (dm2, c), free (dq2:12, h:24, w:24)]
  Bt [128 = (dm2, c), free (j':12, hb:2, wb:2, pi:PCAP)]  stage-1 results,
      j' normalized per proposal (j' = dq2 - sd//2)
  W  [128, scratch]   DVE fold scratch
  U[k] [64 = (c), free (pi:PCAP, db:2, hb:2, wb:2)]  d-slice layer k
  V  [64 = (c), same free layout]  final output -- pi-major, so the two
      stores read contiguous runs (large DMA descriptors)

Stage 1 (DVE): fused TensorReduce over (lh x lw) keeping (j', hb, wb).
For big crops with even nw (or nh) one half-fold tensor_tensor level (bf16
2x mode, 0.52 ns/elem vs TensorReduce's mode-less 1.04) runs first, then a
smaller TensorReduce.  1x1-bin crops are plain Pool copies.  Execution
order: input-arrival gate, then big-ld first.

Stage 2 (no DMA shuffle; walrus only allows tensor-tensor max on DVE, and
only with equal input partition bases): a d-bin of a proposal maps to one
d-slice per (bin, k) with lane dm2=(r+s2)%2, so Pool stages the k-th slice
of every (run, bin) into layer tile U[k] at partition base 0 (cross-base
1-input copies are legal); slice 0 goes straight into V.  pi columns are
sorted by ascending ld = (nd+1)//2 in three chunks [small-ld early |
big-ld | late small-ld (tail store region)], and ONE in-place DVE
tensor_tensor per level k accumulates U[k] into V over the contiguous
SUFFIX of proposals with ld > k -- one instruction per level for the whole
core amortizes DVE's ~60ns fixed cost, and high levels touch only the
trailing columns.  The output leaves in a bulk store plus a tiny tail
store covering the last TAILN columns (the late chunk).

The host-side planner (_plan_core) models engine clocks and emits all ops
in modeled start order; the tile framework infers exact semaphores from
program order.  DMAs stay outside the per-core branch arms (conditional
DMAs break walrus queue-sem watermarks).

bf16 end-to-end: max() commutes with monotone rounding, so the result
equals round_bf16(exact f32 result); rel err <= 2^-8 ~ 4e-3 within the
2e-2 gate.
"""

import os
import sys

import numpy as np

for _p in ("/opt/trn_rl_repo", os.path.expanduser("~/.axon_site/_ro/trn_rl_repo")):
    if os.path.isdir(_p) and _p not in sys.path:
        sys.path.insert(0, _p)

import ml_dtypes  # noqa: E402

import concourse.bass as bass  # noqa: E402
import concourse.tile as tile  # noqa: E402
import concourse.tile_sem_assignment as _tsa  # noqa: E402
from concourse import mybir  # noqa: E402
from concourse.bass_utils import run_bass_kernel_spmd  # noqa: E402

# Walrus encodes at most ONE sync-wait per compute instruction.  Pin HWDGE
# completion-sem lanes by issuing engine so every consumer waits on one
# per-queue FIFO sem; extra waits are split off by _hoist_extra_waits.
if not getattr(_tsa, "_ant_engine_keyed_lanes", False):
    _orig_assign_tick = _tsa.TileClockTick._assign_tick

    def _assign_tick_engine_lanes(self, inst):
        if isinstance(inst, _tsa.DMAInst) and inst.engine in (
            mybir.EngineType.SP,
            mybir.EngineType.Activation,
        ):
            self.next_hw_dma_idx = 0 if inst.engine == mybir.EngineType.SP else 1
        return _orig_assign_tick(self, inst)

    _tsa.TileClockTick._assign_tick = _assign_tick_engine_lanes
    _tsa._ant_engine_keyed_lanes = True

B, P, C, S = 4, 64, 64, 24
JS = 12
BF = mybir.dt.bfloat16

# cost-model constants (ns)
DVE_EL = 1.0417
DVE_FIX = 60.4
DVE_EL2 = 0.5208     # bf16 packed tensor_tensor (2x mode)
POOL_CP = 0.45       # Pool tensor_copy ns/elem (observed rate)
POOL_OP = 3.0
SEM_HOP = 100.0
CHUNK_CUTS = (700.0,)  # B/C boundary, ns before the last s1 finish
TAILN = 8            # pi columns in the tiny trailing store
LD_A = 4             # ld >= LD_A proposals form the early long-chain chunk

# input DMA plan: per queue, list of (first_slab, n_slabs)
IN_PLAN = {
    "SP": [(0, 1), (3, 2), (9, 1)],
    "Act": [(1, 1), (5, 2), (10, 1)],
    "Pool": [(2, 1), (7, 2), (11, 1)],
}


def _bin_params(corners: np.ndarray):
    """Replicate the reference's float32 bin math. Returns lo, n int64 [B,P,3]
    with axis order (d, h, w)."""
    c = corners.astype(np.float32) / np.float32(4.0)
    LL = np.clip(c[:, :, 0, :], np.float32(0.0), np.float32(21.0))
    UR = c[:, :, 1, :]
    UR = np.where(UR - LL >= np.float32(2.0), UR, LL + np.float32(2.0))
    UR = np.clip(UR, np.float32(2.0), np.float32(23.0))
    lo = np.floor(LL).astype(np.int64)
    n = np.floor(UR).astype(np.int64) - lo
    return lo, n


def _geom(lo_b, n_b, p):
    nd, nh, nw = (int(n_b[p, a]) for a in range(3))
    sd, sh, sw = (int(lo_b[p, a]) for a in range(3))
    lh, lw = (nh + 1) // 2, (nw + 1) // 2
    dq0 = sd // 2
    ndq = (sd + nd - 1) // 2 - dq0 + 1
    return nd, nh, nw, sd, sh, sw, lh, lw, dq0, ndq


def _dma_cost(n_slabs):
    tr = 128 / 16.0 * (n_slabs * 1152 / 22.5)
    return max(500.0, tr + 70.0)


def _slab_vis():
    delays = {"SP": 1717.0, "Act": 1717.0, "Pool": 1883.0}
    t0 = {"SP": 200.0, "Act": 200.0, "Pool": 100.0}
    vis = {}
    for q, dmas in IN_PLAN.items():
        t = t0[q]
        for (s0, cnt) in dmas:
            t += _dma_cost(cnt)
            for s in range(s0, s0 + cnt):
                vis[s] = t + delays[q]
    assert sorted(vis) == list(range(JS)), vis
    return vis


def _pool_busy_until():
    return 100.0 + sum(_dma_cost(c) for _, c in IN_PLAN["Pool"])


def _s1_kind(g):
    """Returns (kind, modeled ns).  kind: 'cp' (Pool), 'tr', 'foldy', 'foldx'."""
    nd, nh, nw, sd, sh, sw, lh, lw, dq0, ndq = g
    K = ndq * 4 * lh * lw
    if lh == 1 and lw == 1:
        return "cp", max(POOL_OP, ndq * 4 * POOL_CP)
    ni = 2 if (ndq > 1 and lh > 1 and lw > 1) else 1
    best = ("tr", ni * DVE_FIX + K * DVE_EL)
    if nw % 2 == 0 and lh > 1 and lw > 1:
        hy = (lh + 1) // 2
        c = 2 * (DVE_FIX + ndq * hy * nw * DVE_EL2) \
            + 2 * (DVE_FIX + ndq * 2 * hy * lw * DVE_EL)
        if c < best[1]:
            best = ("foldy", c)
    if nh % 2 == 0 and lh > 1 and lw > 1:
        hx = (lw + 1) // 2
        c = 2 * (DVE_FIX + ndq * nh * hx * DVE_EL2) \
            + 2 * (DVE_FIX + ndq * 2 * lh * hx * DVE_EL)
        if c < best[1]:
            best = ("foldx", c)
    return best


def _ld_of(nd):
    return (nd + 1) // 2


def _lane_slots(nd, s2, db):
    """[(lane, j0, nj)] with nj>0 for bin db of a (nd, s2) class."""
    ld = _ld_of(nd)
    a = db * (nd // 2)
    b = a + ld
    out = []
    for l in (0, 1):
        r0 = a + ((l - s2 - a) % 2)
        if r0 < b:
            nj = (b - r0 + 1) // 2
            out.append((l, (r0 - l + s2) // 2, nj))
    return out


def _slice_of(nd, s2, db, k):
    """(lane, j') of the k-th d-slice of bin db."""
    r = db * (nd // 2) + k
    l = (r + s2) % 2
    return l, (r - l + s2) // 2


def _pi_order(lo_b, n_b, props, chunk_of=None):
    # ascending ld within a chunk: accumulation level k covers the SUFFIX
    # of proposals with ld > k, so each chunk's last-written columns sit at
    # its end (next to the store-tail region).
    key = lambda p: ((chunk_of[p] if chunk_of else 0),
                     _ld_of(int(n_b[p, 0])), int(n_b[p, 0]),
                     int(lo_b[p, 0]) % 2, int(lo_b[p, 0]), p)
    return sorted(props, key=key)


GB_MIN_NET = 50.0


def _pick_families(geom, kind):
    """Batch small crops of one core into padded family cells so ONE
    TensorReduce per family replaces per-proposal ones.  x-type families
    fix lh == LH and pad lw up to w; y-type fix lw == LW and pad lh."""
    fams = []
    taken = set()
    specs = [("x", LH) for LH in (1, 2, 3)] + [("y", LW) for LW in (1, 2)]
    for fam, fixed in specs:
        cands = []
        for p, g in geom.items():
            if p in taken or kind[p][0] == "cp":
                continue
            lh, lw, ndq = g[6], g[7], g[9]
            if fam == "x":
                if lh != fixed or lw < 2:
                    continue
                cands.append((p, lw, ndq))
            else:
                if lw != fixed or lh < 2:
                    continue
                cands.append((p, lh, ndq))
        if len(cands) < 3:
            continue
        best = None
        for w in sorted({ln for _, ln, _ in cands}):
            mem = []
            net = -DVE_FIX
            for p, ln, ndq in cands:
                if ln > w:
                    continue
                m = kind[p][1] - ndq * 4 * fixed * w * DVE_EL
                if m > 15.0:
                    mem.append((p, ln, ndq))
                    net += m
            if len(mem) >= 3 and (best is None or net > best[0]):
                best = (net, w, mem)
        if best is None or best[0] < GB_MIN_NET:
            continue
        net, w, mem = best
        slot0, members = 0, []
        for p, ln, ndq in sorted(mem):
            members.append((p, slot0))
            slot0 += ndq
            taken.add(p)
        fams.append(dict(fam=fam, w=w, fixed=fixed, members=members, ns=slot0))
    return fams


_split_cache = {}


def _split_assign(lo_b, n_b):
    """Split 64 proposals into two halves minimizing the max modeled end."""
    key = (lo_b.tobytes(), n_b.tobytes())
    if key in _split_cache:
        return _split_cache[key]
    costs = [(_s1_kind(_geom(lo_b, n_b, p))[1], p) for p in range(P)]
    costs.sort(key=lambda t: (-t[0], t[1]))
    halves, tot = ([], []), [0.0, 0.0]
    for c, p in costs:
        h = 0 if tot[0] <= tot[1] else 1
        halves[h].append(p)
        tot[h] += c

    def end(props):
        return _plan_core(lo_b, n_b, props, P)["model_end"]

    e = [end(halves[0]), end(halves[1])]
    for _ in range(8):
        d = 0 if e[0] >= e[1] else 1
        best = (max(e), None)
        for p in halves[d]:
            h0 = [x for x in halves[d] if x != p]
            h1 = halves[1 - d] + [p]
            ne = (end(h0), end(h1))
            if max(ne) < best[0] - 1.0:
                best = (max(ne), (p, ne))
        if best[1] is None:
            break
        p, ne = best[1]
        halves[d].remove(p)
        halves[1 - d].append(p)
        e = [ne[0], ne[1]] if d == 0 else [ne[1], ne[0]]
    res = (sorted(halves[0]), sorted(halves[1]))
    _split_cache[key] = res
    return res


def _runs_of(props, lo_b, n_b):
    i = 0
    while i < len(props):
        nd = int(n_b[props[i], 0])
        s2 = int(lo_b[props[i], 0]) % 2
        m = 1
        while (i + m < len(props)
               and int(n_b[props[i + m], 0]) == nd
               and int(lo_b[props[i + m], 0]) % 2 == s2):
            m += 1
        yield i, m, nd, s2
        i += m


def _plan_core(lo_b, n_b, props, pcap):
    """Schedule one core.  Returns pi columns and one merged ordered op list."""
    vis = _slab_vis()
    geom = {p: _geom(lo_b, n_b, p) for p in props}

    def gate(p):
        g = geom[p]
        dq0, ndq = g[8], g[9]
        return max(vis[q] for q in range(dq0, dq0 + ndq))

    kind = {p: _s1_kind(geom[p]) for p in props}
    fams = _pick_families(geom, kind)
    gb = {}           # p -> (fam_idx, slot0)
    for fi, f in enumerate(fams):
        for p, s0 in f["members"]:
            gb[p] = (fi, s0)
    goff, g2off = [], []
    gsz = g2sz = 0
    for f in fams:
        goff.append(gsz)
        g2off.append(g2sz)
        gsz += f["ns"] * 4 * f["w"] * f["fixed"]
        g2sz += f["ns"] * 4

    TD, TP = 0.0, _pool_busy_until()
    s1_fin = {}
    s1_sched = []
    gb_ops = []       # (start, op) for Pool/DVE family ops
    if fams:
        c = max(POOL_OP, gsz * 0.833)
        gb_ops.append((TP, dict(kind="gmem", size=gsz)))
        TP += c
        frel = [0.0] * len(fams)
        gmem_done = TP
        for p in sorted(gb, key=gate):
            fi, s0 = gb[p]
            f = fams[fi]
            nq = 2 if f["fixed"] == 1 else 4
            cg = max(POOL_OP, geom[p][9] * 4 * (geom[p][6] * geom[p][7]) * POOL_CP)
            for q in range(nq):
                st = max(TP, gate(p), gmem_done)
                TP = st + cg / nq
                gb_ops.append((st, dict(kind="gath", p=p, q=q)))
            frel[fi] = max(frel[fi], TP + SEM_HOP)

    # stage-1 priority: arrival gate, then big-ld first (so the trailing
    # accumulation chains belong to small-ld chunks), then big cost first
    tasks = [(gate(p), -_ld_of(int(n_b[p, 0])), -kind[p][1], "p", p)
             for p in props if p not in gb]
    for fi, f in enumerate(fams):
        cost = DVE_FIX + f["ns"] * 4 * f["w"] * f["fixed"] * DVE_EL
        tasks.append((frel[fi], 0, -cost, "f", fi))
    tasks.sort()
    for g0k, _, negc, tk, x in tasks:
        if tk == "f":
            f = fams[x]
            st = max(TD, g0k)
            TD = st + (DVE_FIX + f["ns"] * 4 * f["w"] * f["fixed"] * DVE_EL)
            gb_ops.append((st, dict(kind="ctr", fi=x)))
            for p, s0 in f["members"]:
                cb = max(POOL_OP, geom[p][9] * 8 * POOL_CP)
                stc = max(TP, TD + SEM_HOP)
                TP = stc + cb
                gb_ops.append((stc, dict(kind="cback", p=p)))
                s1_fin[p] = TP
            continue
        p = x
        k, c = kind[p]
        if k == "cp":
            st = max(TP, g0k)
            TP = st + c
            s1_fin[p] = TP
        else:
            st = max(TD, g0k)
            TD = st + c
            s1_fin[p] = TD
        s1_sched.append((st, p, k))

    # chunks: 0 = small-ld early, 1 = big-ld (placed after so ld ascends
    # over chunks 0+1 and ONE merged suffix-level chain serves both; its
    # high levels touch only the trailing columns next to the tail store),
    # 2 = last-finishing small-ld proposals, capped to the tail region.
    fmax = max(s1_fin.values())
    cut = CHUNK_CUTS[-1]

    def _chunk(p):
        if _ld_of(int(n_b[p, 0])) >= LD_A:
            return 1
        return 2 if s1_fin[p] > fmax - cut else 0
    chunk_of = {p: _chunk(p) for p in props}
    last = sorted([p for p in props if chunk_of[p] == 2],
                  key=lambda p: s1_fin[p])
    for p in last[:-TAILN] if len(last) > TAILN else []:
        chunk_of[p] = 0
    ncut = 2
    if all(v == ncut for v in chunk_of.values()):
        chunk_of = {p: 0 for p in props}
    pi_order = _pi_order(lo_b, n_b, props, chunk_of)
    # pi columns: front chunks pack from 0; the last chunk is pinned to the
    # final TAILN-capped columns so the split store's tail is uniform.
    n_last = sum(1 for p in props if chunk_of[p] == ncut)
    n_front = len(props) - n_last
    col = {}
    for i, p in enumerate(pi_order):
        col[p] = i if i < n_front else pcap - n_last + (i - n_front)
    pi_of = col

    dve_ops = []
    pool_ops = []
    for st, p, k in s1_sched:
        if k == "cp":
            pool_ops.append((st, dict(kind="s1cp", p=p, pi=pi_of[p])))
        else:
            dve_ops.append((st, dict(kind="s1", p=p, pi=pi_of[p], s1kind=k)))

    # ---- stage-2 ----
    rinfo = []
    i = 0
    while i < len(pi_order):
        nd = int(n_b[pi_order[i], 0])
        s2 = int(lo_b[pi_order[i], 0]) % 2
        m = 1
        while (i + m < len(pi_order)
               and int(n_b[pi_order[i + m], 0]) == nd
               and int(lo_b[pi_order[i + m], 0]) % 2 == s2
               and col[pi_order[i + m]] == col[pi_order[i]] + m):
            m += 1
        members = pi_order[i:i + m]
        hop = SEM_HOP if any(kind[p][0] != "cp" for p in members) else 0.0
        ready = max(s1_fin[p] for p in members) + hop
        rinfo.append(dict(i0=col[pi_order[i]], m=m, nd=nd, s2=s2,
                          ld=_ld_of(nd), ready=ready,
                          chunk=chunk_of[pi_order[i]]))
        i += m

    # Pool slice copies: k-th d-slice of each (run, bin); k==0 goes straight
    # to V, k>=1 to layer tile U[k].  Track per-(chunk, k) completion.
    layer_done = {}
    for r in sorted(rinfo, key=lambda r: r["ready"]):
        i0, m, nd, s2, ld = r["i0"], r["m"], r["nd"], r["s2"], r["ld"]
        c1 = max(POOL_OP, m * 4 * POOL_CP)
        for k in range(ld):
            for db in range(2):
                l, j = _slice_of(nd, s2, db, k)
                st = max(TP, r["ready"])
                TP = st + c1
                pool_ops.append((st, dict(kind="slice", i0=i0, m=m, db=db,
                                          k=k, lane=l, j0=j)))
            key = (r["chunk"], k)
            layer_done[key] = max(layer_done.get(key, 0.0), TP)

    # DVE level accumulations: one merged chain over chunks 0+1 (ld ascends
    # across both), a separate short chain for the tail chunk 2.  Level k
    # covers the SUFFIX of proposals with ld > k.
    for cids in ((0, 1), (2,)):
        cprops = [p for p in pi_order if chunk_of[p] in cids]
        if not cprops:
            continue
        cend = max(col[p] for p in cprops) + 1
        lds = [_ld_of(int(n_b[p, 0])) for p in cprops]
        ldmax = max(lds)
        prev = 0.0
        for k in range(1, ldmax):
            span = sum(1 for x in lds if x > k)
            rel = prev
            for cid in cids:
                rel = max(rel, layer_done.get((cid, k), 0.0) + SEM_HOP,
                          layer_done.get((cid, 0), 0.0) + SEM_HOP)
            st = max(TD, rel)
            TD = st + DVE_FIX + span * 8 * DVE_EL2
            prev = TD
            dve_ops.append((st, dict(kind="acc", g0=cend - span, span=span,
                                     k=k)))
    model_end = max(TD, TP)
    # Merge to one stream ordered by modeled start: tile deps are inferred
    # from program order, so every consumer must be emitted after its
    # producer (consumer start >= producer finish guarantees this).
    s1k = ("s1", "s1cp", "gmem", "gath", "ctr", "cback")
    allops = [(st, 0 if o["kind"] in s1k else 1, i, o)
              for i, (st, o) in enumerate(dve_ops + pool_ops + gb_ops)]
    allops.sort(key=lambda t: (t[0], t[1], t[2]))
    return dict(columns={p: col[p] for p in props}, pi_order=pi_order,
                ops=[o for _, _, _, o in allops],
                n_front=n_front, n_last=n_last, fams=fams, gb=gb,
                goff=goff, g2off=g2off, gsz=gsz, g2sz=g2sz,
                nprops=len(props), model_end=model_end)


def _mk_ap(base_ap, extra_offset: int, dims):
    ap = base_ap.copy()
    ap.ap = mybir.VecI64Pair([list(base_ap.ap[0])] + [[s, c] for s, c in dims])
    ap.offset = base_ap.offset + extra_offset
    return ap


def _switch8(tc, bid, hid):
    for b in range(4):
        with tc.If(bid == b):
            for half in range(2):
                with tc.If(hid == half):
                    yield 2 * b + half


def _hoist_extra_waits(nc: bass.Bass) -> None:
    """Split all but one sync-wait of any multi-wait instruction onto
    standalone EventSemaphore ops just before it (same in-order engine
    stream) -- walrus allows at most one wait per instruction."""
    for bb in nc.m.functions[0].blocks:
        insts = bb.instructions
        i = 0
        while i < len(insts):
            ins = insts[i]
            si = ins.sync_info
            if si is not None and len(si.on_wait) > 1 and ins.opcode != "EventSemaphore":
                waits = list(si.on_wait)
                for j, w in enumerate(waits[:-1]):
                    insts.insert(
                        i + j,
                        mybir.InstEventSemaphore(
                            name=f"{ins.name}_w{j}",
                            engine=ins.engine,
                            ins=[], outs=[],
                            sync_info=mybir.SyncInfo(on_wait=[w], on_update=[]),
                        ),
                    )
                si.on_wait = [waits[-1]]
                i += len(waits) - 1
            i += 1


def _layout_caps(lo, n):
    pc, ldm = 1, 2
    for b in range(B):
        for half in _split_assign(lo[b], n[b]):
            pc = max(pc, len(half))
        ldm = max(ldm, max(_ld_of(int(x)) for x in n[b][:, 0]))
    return pc, ldm


def _dims(dd):
    f = [d for d in dd if d[1] > 1]
    return f or [[1, 1]]


def _build_program(lo: np.ndarray, n: np.ndarray, hoist: bool = True) -> bass.Bass:
    nc = bass.Bass("TRN2", target_bir_lowering=False, debug=False, num_devices=8)
    PCAP, LDM = _layout_caps(lo, n)
    J_S, HB_S, WB_S = 4 * PCAP, 2 * PCAP, PCAP        # Bt strides
    # V/U layout is pi-major (pi:8, db:4, hb:2, wb:1) so stores read
    # contiguous runs (big DMA descriptors) while acc TTs keep a packed
    # innermost dim for the bf16 2x mode.

    fm_d = nc.dram_tensor("fm_part", [128, JS * S * S], BF, kind="ExternalInput")
    out_d = nc.dram_tensor("out_v", [64, 8 * PCAP], BF, kind="ExternalOutput")

    splits = [_split_assign(lo[b], n[b]) for b in range(B)]
    plans = [[_plan_core(lo[b], n[b], splits[b][h], PCAP) for h in (0, 1)]
             for b in range(B)]

    with tile.TileContext(nc) as tc:
        with tc.tile_pool(name="pool", bufs=1) as pool:
            A = pool.tile([128, JS * S * S], BF)
            Bt = pool.tile([128, JS * 4 * PCAP], BF)
            W = pool.tile([128, 4096], BF)
            U = [pool.tile([64, 8 * PCAP], BF, name=f"U{i}")
                 for i in range(LDM)]
            V = pool.tile([64, 8 * PCAP], BF)
            GCAP = max([1] + [pl["gsz"] for bb_ in plans for pl in bb_])
            G2CAP = max([1] + [pl["g2sz"] for bb_ in plans for pl in bb_])
            G = pool.tile([128, GCAP], BF)
            G2 = pool.tile([128, G2CAP], BF)

            pid = nc.partition_id(
                engines=(mybir.EngineType.DVE, mybir.EngineType.Pool)
            )
            bid = pid >> 1
            hid = pid & 1

            qeng = {"SP": nc.sync, "Act": nc.scalar, "Pool": nc.gpsimd}
            for q, dmas in IN_PLAN.items():
                for (s0, cnt) in dmas:
                    src = fm_d.ap().copy()
                    src.ap = mybir.VecI64Pair(
                        [[JS * S * S, 128], [1, cnt * S * S]]
                    )
                    src.offset = s0 * S * S
                    qeng[q].dma_start(
                        _mk_ap(A[:], s0 * S * S, [[1, cnt * S * S]]), src)

            def emit_s1_tr(g, pi):
                nd, nh, nw, sd, sh, sw, lh, lw, dq0, ndq = g
                base_in = dq0 * (S * S) + sh * S + sw
                red = ([[S, lh]] if lh > 1 else []) + (
                    [[1, lw]] if lw > 1 else [])
                nred = len(red)
                ax = mybir.AxisListType.XY if nred == 2 else mybir.AxisListType.X
                if (1 if ndq > 1 else 0) + 2 + nred <= 4:
                    din = [[(nh // 2) * S, 2], [nw // 2, 2]]
                    dout = [[HB_S, 2], [WB_S, 2]]
                    if ndq > 1:
                        din.insert(0, [S * S, ndq])
                        dout.insert(0, [J_S, ndq])
                    nc.vector.tensor_reduce(
                        out=_mk_ap(Bt[:], pi, dout),
                        in_=_mk_ap(A[:], base_in, din + red),
                        axis=ax, op=mybir.AluOpType.max)
                else:
                    for hb in range(2):
                        nc.vector.tensor_reduce(
                            out=_mk_ap(Bt[:], pi + hb * HB_S,
                                       [[J_S, ndq], [WB_S, 2]]),
                            in_=_mk_ap(A[:], base_in + hb * (nh // 2) * S,
                                       [[S * S, ndq], [nw // 2, 2]] + red),
                            axis=ax, op=mybir.AluOpType.max)

            def emit_s1_foldy(g, pi):
                # even nw, lh > 1: one y-half fold (TT, 2x) per hb into W,
                # then one 4-dim TensorReduce per hb into Bt.
                nd, nh, nw, sd, sh, sw, lh, lw, dq0, ndq = g
                base_in = dq0 * (S * S) + sh * S + sw
                hy = (lh + 1) // 2
                WY, WJ, WHB = nw, nw * hy, nw * hy * ndq
                for hb in range(2):
                    o0 = base_in + hb * (nh // 2) * S
                    din = _dims([[S * S, ndq], [S, hy], [1, nw]])
                    nc.vector.tensor_tensor(
                        out=_mk_ap(W[:], hb * WHB,
                                   _dims([[WJ, ndq], [WY, hy], [1, nw]])),
                        in0=_mk_ap(A[:], o0, din),
                        in1=_mk_ap(A[:], o0 + (lh - hy) * S, din),
                        op=mybir.AluOpType.max)
                for hb in range(2):
                    nc.vector.tensor_reduce(
                        out=_mk_ap(Bt[:], pi + hb * HB_S,
                                   [[J_S, ndq], [WB_S, 2]]),
                        in_=_mk_ap(W[:], hb * WHB,
                                   [[WJ, ndq], [lw, 2], [WY, hy], [1, lw]]),
                        axis=mybir.AxisListType.XY, op=mybir.AluOpType.max)

            def emit_s1_foldx(g, pi):
                # even nh, lw > 1: one x-half fold (TT, 2x) per wb into W
                # (h-rows merged over both bins), then one TensorReduce per
                # hb into Bt.  W layout: (j', rows:nh, wb:2, x:hx).
                nd, nh, nw, sd, sh, sw, lh, lw, dq0, ndq = g
                base_in = dq0 * (S * S) + sh * S + sw
                hx = (lw + 1) // 2
                WR, WWBs, WJ = hx, 0, 0  # row stride hx, wb stride set below
                WWBs = nh * hx
                WJ = 2 * nh * hx
                for wb in range(2):
                    o0 = base_in + wb * (nw // 2)
                    din = _dims([[S * S, ndq], [S, nh], [1, hx]])
                    nc.vector.tensor_tensor(
                        out=_mk_ap(W[:], wb * WWBs,
                                   _dims([[WJ, ndq], [WR, nh], [1, hx]])),
                        in0=_mk_ap(A[:], o0, din),
                        in1=_mk_ap(A[:], o0 + (lw - hx), din),
                        op=mybir.AluOpType.max)
                for hb in range(2):
                    nc.vector.tensor_reduce(
                        out=_mk_ap(Bt[:], pi + hb * HB_S,
                                   [[J_S, ndq], [WB_S, 2]]),
                        in_=_mk_ap(W[:], hb * (nh // 2) * WR,
                                   [[WJ, ndq], [WWBs, 2], [WR, lh], [1, hx]]),
                        axis=mybir.AxisListType.XY, op=mybir.AluOpType.max)

            def emit_s1_cp(g, pi):
                nd, nh, nw, sd, sh, sw, lh, lw, dq0, ndq = g
                base_in = dq0 * (S * S) + sh * S + sw
                nc.gpsimd.tensor_copy(
                    _mk_ap(Bt[:], pi, _dims([[J_S, ndq], [HB_S, 2],
                                             [WB_S, 2]])),
                    _mk_ap(A[:], base_in,
                           _dims([[S * S, ndq], [(nh // 2) * S, 2],
                                  [nw // 2, 2]])))

            def v_slot(T, i0, m, db):
                # dims ordered (pi, hb, wb) to pair with bt_slot
                return _mk_ap(T[:], i0 * 8 + db * 4,
                              _dims([[8, m], [2, 2], [1, 2]]))

            def bt_slot(lane, j0, i0, m):
                return _mk_ap(Bt[lane * 64:(lane + 1) * 64], j0 * J_S + i0,
                              _dims([[1, m], [HB_S, 2], [WB_S, 2]]))

            for case in _switch8(tc, bid, hid):
                b, half = case // 2, case % 2
                plan = plans[b][half]
                lo_b, n_b = lo[b], n[b]
                geom = {p: _geom(lo_b, n_b, p) for p in plan["pi_order"]}
                gap0, gap1 = plan["n_front"], PCAP - plan["n_last"]
                if gap0 < gap1:
                    nc.gpsimd.memset(
                        _mk_ap(V[:], gap0 * 8, [[1, (gap1 - gap0) * 8]]), 0)
                for op in plan["ops"]:
                    k = op["kind"]
                    if k == "gmem":
                        nc.gpsimd.memset(
                            _mk_ap(G[:], 0, [[1, plan["gsz"]]]), -3.0e38)
                    elif k == "gath":
                        p, q = op["p"], op["q"]
                        fi, s0 = plan["gb"][p]
                        f = plan["fams"][fi]
                        nd, nh, nw, sd, sh, sw, lh, lw, dq0, ndq = geom[p]
                        fx, w = f["fixed"], f["w"]
                        cellq = fx * w
                        if fx == 1:
                            hb, wb = q, None
                        else:
                            hb, wb = q // 2, q % 2
                        base_in = dq0 * S * S + sh * S + sw \
                            + hb * (nh // 2) * S
                        base_g = plan["goff"][fi] + s0 * 4 * cellq \
                            + hb * 2 * cellq
                        if wb is not None:
                            base_in += wb * (nw // 2)
                            base_g += wb * cellq
                        if f["fam"] == "x":
                            if fx == 1:
                                out = _mk_ap(G[:], base_g,
                                             _dims([[4 * cellq, ndq],
                                                    [cellq, 2], [1, lw]]))
                                din = _mk_ap(A[:], base_in,
                                             _dims([[S * S, ndq],
                                                    [nw // 2, 2], [1, lw]]))
                            else:
                                out = _mk_ap(G[:], base_g,
                                             _dims([[4 * cellq, ndq],
                                                    [w, fx], [1, lw]]))
                                din = _mk_ap(A[:], base_in,
                                             _dims([[S * S, ndq],
                                                    [S, fx], [1, lw]]))
                        else:
                            if fx == 1:
                                out = _mk_ap(G[:], base_g,
                                             _dims([[4 * cellq, ndq],
                                                    [1, lh], [cellq, 2]]))
                                din = _mk_ap(A[:], base_in,
                                             _dims([[S * S, ndq], [S, lh],
                                                    [nw // 2, 2]]))
                            else:
                                out = _mk_ap(G[:], base_g,
                                             _dims([[4 * cellq, ndq],
                                                    [fx, lh], [1, fx]]))
                                din = _mk_ap(A[:], base_in,
                                             _dims([[S * S, ndq], [S, lh],
                                                    [1, fx]]))
                        nc.gpsimd.tensor_copy(out, din)
                    elif k == "ctr":
                        f = plan["fams"][op["fi"]]
                        cellq = f["fixed"] * f["w"]
                        ns = f["ns"]
                        go, g2o = plan["goff"][op["fi"]], plan["g2off"][op["fi"]]
                        nc.vector.tensor_reduce(
                            out=_mk_ap(G2[:], g2o, [[1, ns * 4]]),
                            in_=_mk_ap(G[:], go, [[cellq, ns * 4], [1, cellq]]),
                            axis=mybir.AxisListType.X,
                            op=mybir.AluOpType.max)
                    elif k == "cback":
                        p = op["p"]
                        fi, s0 = plan["gb"][p]
                        ndq = geom[p][9]
                        pi = plan["columns"][p]
                        nc.gpsimd.tensor_copy(
                            _mk_ap(Bt[:], pi, _dims([[J_S, ndq], [HB_S, 2],
                                                     [WB_S, 2]])),
                            _mk_ap(G2[:], plan["g2off"][fi] + s0 * 4,
                                   _dims([[4, ndq], [2, 2], [1, 2]])))
                    elif k == "s1cp":
                        emit_s1_cp(geom[op["p"]], op["pi"])
                    elif k == "slice":
                        dst = V if op["k"] == 0 else U[op["k"]]
                        nc.gpsimd.tensor_copy(
                            v_slot(dst, op["i0"], op["m"], op["db"]),
                            bt_slot(op["lane"], op["j0"], op["i0"], op["m"]))
                    elif k == "s1":
                        sk = op["s1kind"]
                        if sk == "foldy":
                            emit_s1_foldy(geom[op["p"]], op["pi"])
                        elif sk == "foldx":
                            emit_s1_foldx(geom[op["p"]], op["pi"])
                        else:
                            emit_s1_tr(geom[op["p"]], op["pi"])
                    else:
                        g0, span = op["g0"], op["span"]
                        dims = _dims([[1, span * 8]])
                        vap = _mk_ap(V[:], g0 * 8, dims)
                        nc.vector.tensor_tensor(
                            out=vap, in0=vap,
                            in1=_mk_ap(U[op["k"]][:], g0 * 8, dims),
                            op=mybir.AluOpType.max)
            # Stores outside the per-core arms: conditional DMAs break the
            # tile framework's queue-sem watermark compensation.  The bulk
            # store excludes the TAILN tail columns, which hold the final
            # chunk's proposals and go out in a second, tiny store.
            nf = PCAP - TAILN
            nc.sync.dma_start(
                _mk_ap(out_d.ap(), 0, [[1, nf * 8]]),
                _mk_ap(V[:], 0, [[1, nf * 8]]))
            nc.scalar.dma_start(
                _mk_ap(out_d.ap(), nf * 8, [[1, TAILN * 8]]),
                _mk_ap(V[:], nf * 8, [[1, TAILN * 8]]))

    if hoist:
        _hoist_extra_waits(nc)
    return nc


def _prep_fm(fm_bf: np.ndarray):
    """fm[b] [C,24,24,24] bf16 -> [128=(dm2,c), 12*576] host pre-transpose."""
    x = fm_bf.reshape(C, JS, 2, S * S).transpose(2, 0, 1, 3)
    return np.ascontiguousarray(x.reshape(2 * C, JS * S * S))


def _unpack(res_k, lo_b, n_b, half):
    props = _split_assign(lo_b, n_b)[half]
    rv = np.asarray(res_k["out_v"]).astype(np.float32)
    PCAP = rv.shape[1] // 8
    cols = _plan_core(lo_b, n_b, props, PCAP)["columns"]
    rv = rv.reshape(C, PCAP, 2, 2, 2)      # [c, pi, db, hb, wb]
    return {p: rv[:, cols[p]] for p in props}


def _run(fm: np.ndarray, corners: np.ndarray, trace: bool = False, trace_cores=None):
    fm = np.asarray(fm, dtype=np.float32)
    corners = np.asarray(corners, dtype=np.float32)
    assert fm.shape == (B, C, S, S, S) and corners.shape == (B, P, 2, 3)

    lo, n = _bin_params(corners)
    nc = _build_program(lo, n)

    fm_bf = fm.astype(ml_dtypes.bfloat16)
    parts = [_prep_fm(fm_bf[b]) for b in range(B)]
    in_maps = [{"fm_part": parts[k // 2]} for k in range(8)]

    res = run_bass_kernel_spmd(
        nc, in_maps, core_ids=list(range(8)), trace=trace,
        **({"trace_cores": trace_cores} if trace_cores else {}),
    )

    out = np.empty((B, P, C, 2, 2, 2), dtype=np.float32)
    for k in range(8):
        b, half = k // 2, k % 2
        vals = _unpack(res.results[k], lo[b], n[b], half)
        for p, v in vals.items():
            out[b, p] = v
    return out, res


def kernel(fm: np.ndarray, corners: np.ndarray) -> np.ndarray:
    out, _ = _run(fm, corners)
    return out
